# revision 36
# baseline (speedup 1.0000x reference)
# Trainium2 Bass kernel for nn_MultiHeadAttention_24902220382931.
#
# Strategy: data-parallel over sentences. The 32 variable-length sentences are
# sorted by length; core c processes ranks {c, 15-c, 16+c, 31-c} (exactly equal
# token counts, near-equal attention work). Each core packs its 4 sentences
# into 4 fixed-size slots (max length per slot across cores, regions rounded to
# 128) so that all 8 cores execute one identical SPMD program. Padded rows are
# zeros; softmax denominators are corrected by subtracting the per-core pad
# count (pad keys contribute exp(0)=1 exactly), shipped as data.
#
# Precision: matmul operands in fp8e4 with DoubleRow perf mode (2x PE rate;
# contraction pairs packed as [K,2,N] APs), attn-transpose in bf16, softmax
# sum / residual / layernorm in fp32. Attention probabilities are scaled x64
# (to keep them in fp8's normal range) and descaled by 2^-6 at the fused
# residual add.
import sys

for _p in ("/opt/trn_rl_repo", "/root/.axon_site/_ro/trn_rl_repo"):
    if _p not in sys.path:
        sys.path.insert(0, _p)

import numpy as np
import ml_dtypes

import concourse.bass as bass  # noqa: F401  (bass types used via bacc/tile)
import concourse.mybir as mybir
import concourse.tile as tile
from concourse import bacc

BF16 = ml_dtypes.bfloat16
FP8 = ml_dtypes.float8_e4m3
F32 = np.float32

N_CORES = 8
MB = 32
D_MODEL = 1024
D_HALF = 512  # d_content == d_pos
N_HEAD = 8
D_K = 128
DK2 = 64
SCALE = float(D_MODEL) ** 0.5  # 32.0
EPS = 1e-3
P = 128  # partitions
ASCALE = 64.0     # attention-probability scale (fp8 normal range)
DESCALE = 1.0 / ASCALE

DR = mybir.MatmulPerfMode.DoubleRow


def _ceil_to(x, m):
    return (x + m - 1) // m * m


class Plan:
    def __init__(self, lengths):
        lengths = np.asarray(lengths, np.int64)
        assert lengths.shape == (MB,)
        order = np.argsort(-lengths, kind="stable")
        # core c handles sentence ranks {c, 15-c, 16+c, 31-c} (desc length order)
        self.core_sents = [
            [int(order[c]), int(order[15 - c]), int(order[16 + c]), int(order[31 - c])]
            for c in range(N_CORES)
        ]
        self.lengths = lengths
        self.slot_pad = [
            max(int(lengths[self.core_sents[c][j]]) for c in range(N_CORES))
            for j in range(4)
        ]
        self.regions = [_ceil_to(sp, P) for sp in self.slot_pad]
        self.offs = [0]
        for r in self.regions[:-1]:
            self.offs.append(self.offs[-1] + r)
        self.t_pad = sum(self.regions)
        assert self.t_pad % P == 0
        self.nt = self.t_pad // P
        self.glob_off = np.concatenate([[0], np.cumsum(lengths)[:-1]]).astype(np.int64)

    @property
    def key(self):
        return (tuple(self.slot_pad), self.t_pad)


def _copy(nc, eng, out, in_):
    # engine-dispatched copy: DVE has tensor_copy, ACT uses activation(Copy)
    if eng is nc.scalar:
        nc.scalar.copy(out, in_)
    else:
        eng.tensor_copy(out, in_)


def _transpose_qi(nc, at_ps, entry, nk):
    # attn^T @ diag(64*recip): transpose + normalize in one matmul per k-chunk
    attn, diag, lq, qoff = entry
    for ki in range(nk):
        nc.tensor.matmul(
            at_ps[ki][0:P, qoff:qoff + lq],
            attn[0:lq, P * ki:P * ki + P],
            diag[0:lq, 0:lq],
            start=True,
            stop=True,
        )


def _build_program(plan: Plan, loop_n: int = 1):
    """Build and compile the single-core Bass program (same for all cores).

    loop_n > 1 wraps the whole computation in a hardware For-loop (for
    steady-state timing measurements; the body is idempotent)."""
    import contextlib
    T = plan.t_pad
    nc = bacc.Bacc("TRN2", target_bir_lowering=False, debug=False)

    dt = mybir.dt
    # ---- DRAM I/O ----
    # xT packed per-slot: [p, s*(8*gw)] with [p, c, t] = x-dim c*128+p of
    # token t -> one contiguous descriptor per partition per slot
    xT_d = nc.dram_tensor("xT", [P, 8 * T], dt.float8e4, kind="ExternalInput").ap()
    x_d = nc.dram_tensor("x", [T, D_MODEL], dt.float32, kind="ExternalInput").ap()
    # weights packed partition-major on host: one contiguous DMA each
    wq_d = nc.dram_tensor("wq", [P, 4 * 8 * P], dt.float8e4, kind="ExternalInput").ap()
    wk_d = nc.dram_tensor("wk", [P, 4 * 8 * P], dt.float8e4, kind="ExternalInput").ap()
    wv_d = nc.dram_tensor("wv", [P, 8 * D_HALF], dt.float8e4, kind="ExternalInput").ap()
    pw_d = nc.dram_tensor("pw", [P, 8 * D_HALF], dt.float8e4, kind="ExternalInput").ap()
    npad_d = nc.dram_tensor("npad", [P, 4], dt.float32, kind="ExternalInput").ap()
    ident_d = nc.dram_tensor("ident", [P, P], dt.bfloat16, kind="ExternalInput").ap()
    out_d = nc.dram_tensor("out", [T, D_MODEL], dt.float32, kind="ExternalOutput").ap()

    with tile.TileContext(nc) as tc:
        with (
            tc.tile_pool(name="persist", bufs=1) as pp,
            tc.tile_pool(name="weights", bufs=1) as wp,
        ):
            # Per-slot persistent tensors (finer dependency granularity lets
            # attention/proj start as soon as a slot's QKV is done).
            # Q^T/K^T pair-stacked: [p, comp(c/p), pair, region]; partition
            # p<64 holds head 2*pair, p>=64 head 2*pair+1 (comp's 64 dims).
            qt = [pp.tile([P, 2, 4, r], dt.float8e4, name=f"qt{s}", tag=f"qt{s}")
                  for s, r in enumerate(plan.regions)]
            kt = [pp.tile([P, 2, 4, r], dt.float8e4, name=f"kt{s}", tag=f"kt{s}")
                  for s, r in enumerate(plan.regions)]
            # V token-natural, head-major columns: [p, tile, head, {c64|p64}]
            vv = [pp.tile([P, r // P, D_MODEL], dt.float8e4, name=f"vv{s}", tag=f"vv{s}")
                  for s, r in enumerate(plan.regions)]
            o1t = [pp.tile([P, 4, r], dt.float8e4, name=f"o1t{s}", tag=f"o1t{s}")
                   for s, r in enumerate(plan.regions)]
            o2t = [pp.tile([P, 4, r], dt.float8e4, name=f"o2t{s}", tag=f"o2t{s}")
                   for s, r in enumerate(plan.regions)]
            npad_sb = pp.tile([P, 4], dt.float32, tag="npad")
            ident_sb = pp.tile([P, P], dt.bfloat16, tag="ident")

            wq_sb = wp.tile([P, 4, 8, P], dt.float8e4, tag="wq")
            wk_sb = wp.tile([P, 4, 8, P], dt.float8e4, tag="wk")
            wv_sb = wp.tile([P, 8, D_HALF], dt.float8e4, tag="wv")
            pw_sb = wp.tile([P, 2, 4, D_HALF], dt.float8e4, tag="pw")

            _c = getattr(plan, "cfg", {})
            aux_eng = getattr(nc, _c.get("aux_eng", "gpsimd"))
            w_eng = getattr(nc, _c.get("w_eng", "scalar"))
            aux_eng.dma_start(npad_sb[:, :], npad_d[:, :])
            aux_eng.dma_start(ident_sb[:, :], ident_d[:, :])
            # per-pr-chunk weight DMAs on separate queues: the first QK
            # matmuls are gated on a small chunk, not the full weight set
            wq_f = wq_sb.rearrange("p a b c -> p a (b c)")
            wk_f = wk_sb.rearrange("p a b c -> p a (b c)")
            wq_r = wq_d.rearrange("p (a r) -> p a r", a=4)
            wk_r = wk_d.rearrange("p (a r) -> p a r", a=4)
            for pr in range(4):
                w_eng.dma_start(wq_f[:, pr, :], wq_r[:, pr, :])
                w_eng.dma_start(wk_f[:, pr, :], wk_r[:, pr, :])
            aux_eng.dma_start(
                wv_sb.rearrange("p a b -> p (a b)")[:, :], wv_d[:, :])
            aux_eng.dma_start(
                pw_sb.rearrange("p a b c -> p (a b c)")[:, :], pw_d[:, :])

            loop_cm = (tc.For_i(0, loop_n, 1,
                                hint_engines=(mybir.EngineType.PE,
                                              mybir.EngineType.DVE,
                                              mybir.EngineType.Activation,
                                              mybir.EngineType.SP))
                       if loop_n > 1 else contextlib.nullcontext())
            with loop_cm:
                _kernel_body(nc, tc, plan, locals())

    nc.compile()
    return nc


def _kernel_body(nc, tc, plan, env):
    dt = mybir.dt
    qt, kt, vv, o1t, o2t = (env["qt"], env["kt"], env["vv"], env["o1t"],
                            env["o2t"])
    npad_sb, ident_sb = env["npad_sb"], env["ident_sb"]
    wq_sb, wk_sb, wv_sb, pw_sb = (env["wq_sb"], env["wk_sb"], env["wv_sb"],
                                  env["pw_sb"])
    xT_d, x_d, out_d = env["xT_d"], env["x_d"], env["out_d"]
    # dr_*: which matmul groups use fp8 DoubleRow. The board power limiter
    # halves the PE clock under sustained full-DR load, so only the QKV
    # projections (+ the half-array logits, which are power-neutral) and the
    # proj run DoubleRow; attn@V stays plain-fp8. Empirically fastest mix.
    cfg = dict(qk=4, v=3, lg=2, at=1, ot=1, z=6,
               dr_qkv=True, dr_lg=True, dr_av=True, dr_proj=False)
    cfg.update(getattr(plan, "cfg", {}))
    out_eng = getattr(nc, cfg.get("out_eng", "gpsimd"))
    aux_eng = getattr(nc, cfg.get("aux_eng", "gpsimd"))
    ms_gp = cfg.get("ms_gp", True)
    ln_gp = cfg.get("ln_gp", True)

    # zero the attention-output staging (pad-query columns are never
    # written; keep them finite for the projection matmuls)
    ms_eng = nc.gpsimd if ms_gp else nc.vector
    for s in range(4):
        L, r = plan.slot_pad[s], plan.regions[s]
        if L < r:
            ms_eng.memset(o1t[s][:, :, L:r], 0.0)
            ms_eng.memset(o2t[s][:, :, L:r], 0.0)

    # ============ Phase 1: QKV projection emitters ============
    # Only slot 0 runs as a dense up-front phase; slots 1-3's QKV matmuls
    # are interleaved as PE filler into the previous slot's attention so
    # the PE never idles long enough for the HAM to re-throttle its clock.
    dr_qkv = cfg.get("dr_qkv", True)

    def _xt_load(xtp, s):
        gw = plan.regions[s]
        g0 = plan.offs[s]
        xt_sb = xtp.tile([P, 8, gw], dt.float8e4, name="xt", tag="xt")
        xt_f2 = xt_sb.rearrange("p a b -> p (a b)")
        # split across several DMA queues (1-1.5KB/partition descriptors)
        for c in range(4):
            nc.sync.dma_start(
                xt_f2[:, 2 * c * gw:2 * (c + 1) * gw],
                xT_d[:, 8 * g0 + 2 * c * gw:8 * g0 + 2 * (c + 1) * gw])
        return xt_sb

    def _qkv_emitters(pool, s, xt_sb, pr_major=False):
        """Closures, each emitting one PSUM acc group (matmuls + copy)."""
        gw = plan.regions[s]
        ems = []

        def _qk_group(half, pr, qk):
            w_sb, dst = ((wq_sb, qt), (wk_sb, kt))[qk]
            acc = pool.tile([P, 512], dt.float32, name="qkacc", tag="lg")
            if dr_qkv:
                for jj in range(2):
                    j0 = half * 4 + 2 * jj
                    nc.tensor.matmul(
                        acc[:, 0:gw],
                        w_sb[:, pr, j0:j0 + 2, :],
                        xt_sb[:, j0:j0 + 2, 0:gw],
                        start=(jj == 0),
                        stop=(jj == 1),
                        perf_mode=DR,
                    )
            else:
                for jj in range(4):
                    j = half * 4 + jj
                    nc.tensor.matmul(
                        acc[:, 0:gw],
                        w_sb[:, pr, j, :],
                        xt_sb[:, j, 0:gw],
                        start=(jj == 0),
                        stop=(jj == 3),
                    )
            eng = nc.vector if (pr + qk + half) % 2 else nc.scalar
            _copy(nc, eng, dst[s][:, half, pr, 0:gw], acc[:, 0:gw])

        vv_w = vv[s].rearrange("p t (h b d) -> p t h b d", h=N_HEAD, b=2)

        def _v_group(tt, half):
            tl = tt * P
            vacc = pool.tile([P, 512], dt.float32, name="vacc", tag="lg")
            vacc_r = vacc.rearrange("p (h d) -> p h d", h=N_HEAD)
            if dr_qkv:
                for jj in range(2):
                    j0 = half * 4 + 2 * jj
                    nc.tensor.matmul(
                        vacc[:, :],
                        xt_sb[:, j0:j0 + 2, tl:tl + P],
                        wv_sb[:, j0:j0 + 2, :],
                        start=(jj == 0),
                        stop=(jj == 1),
                        perf_mode=DR,
                    )
            else:
                for jj in range(4):
                    j = half * 4 + jj
                    nc.tensor.matmul(
                        vacc[:, :],
                        xt_sb[:, j, tl:tl + P],
                        wv_sb[:, j, :],
                        start=(jj == 0),
                        stop=(jj == 3),
                    )
            nc.scalar.copy(vv_w[:, tt, :, half, :], vacc_r[:, :, :])

        if pr_major:
            # up-front slot: pr-major order so head 0's q/k (pr=0, both
            # halves) complete first and attention starts earliest
            for pr in range(4):
                for half in range(2):
                    for qk in range(2):
                        ems.append(lambda h=half, p=pr, q=qk: _qk_group(h, p, q))
            for half in range(2):
                for tt in range(gw // P):
                    ems.append(lambda t=tt, h=half: _v_group(t, h))
        else:
            # half-0 groups first: PE can start while xt half-1 is landing
            for half in range(2):
                for pr in range(4):
                    for qk in range(2):
                        ems.append(lambda h=half, p=pr, q=qk: _qk_group(h, p, q))
                for tt in range(gw // P):
                    ems.append(lambda t=tt, h=half: _v_group(t, h))
        return ems

    # dense up-front QKV for slot 0 in its own (wider) PSUM scope
    with (
        tc.tile_pool(name="xt0_pool", bufs=1) as xtp0,
        tc.tile_pool(name="p1_ps", bufs=7, space="PSUM") as p1ps,
    ):
        xt0 = _xt_load(xtp0, 0)
        for em in _qkv_emitters(p1ps, 0, xt0, pr_major=True):
            em()

    # ======== Phase 2+3: attention + proj/LN + QKV filler ========
    # PSUM: lg/filler(2) + atp0..3(4) + ozp(2, attn-out + proj acc) = 8
    x_dma = nc.sync
    out_dma = out_eng
    with (
        tc.tile_pool(name="xt_pool", bufs=2) as xtp,
        tc.tile_pool(name="lg_ps", bufs=2, space="PSUM") as lgps,
        tc.tile_pool(name="at_ps", bufs=1, space="PSUM") as atps,
        tc.tile_pool(name="ozp_ps", bufs=2, space="PSUM") as ozps,
        tc.tile_pool(name="attn_sb", bufs=4) as asb,
        tc.tile_pool(name="small_sb", bufs=6) as ssb,
        tc.tile_pool(name="z_sb", bufs=3) as zsb,
        tc.tile_pool(name="x_sb", bufs=3) as xsb,
        tc.tile_pool(name="ln_sb", bufs=4) as lsb,
    ):
        def _slot_dims(s):
            gw = plan.regions[s]
            L = plan.slot_pad[s]
            nk = gw // P
            return L, gw, nk

        p3_state = {}  # slot -> (z tiles, mv4 tile)

        def _emit_p3a(s, tt):
            # proj + residual + one-pass LN stats for one token tile
            g0 = plan.offs[s]
            lt = tt * P
            t0 = g0 + lt
            if tt == 0:
                p3_state[s] = ([None] * 4,
                               lsb.tile([P, 4, 2], dt.float32, name="mv4", tag="mv4",
                                        bufs=2))
            zs, mv4 = p3_state[s]
            zh = []
            for i, osrc in enumerate((o1t[s], o2t[s])):
                zp = ozps.tile([P, 512], dt.float32, name="zp", tag="ozp")
                if cfg.get("dr_proj", True):
                    for kp in range(2):
                        nc.tensor.matmul(
                            zp[:, :],
                            osrc[:, 2 * kp:2 * kp + 2, lt:lt + P],
                            pw_sb[:, i, 2 * kp:2 * kp + 2, :],
                            start=(kp == 0),
                            stop=(kp == 1),
                            perf_mode=DR,
                        )
                else:
                    for k in range(4):
                        nc.tensor.matmul(
                            zp[:, :],
                            osrc[:, k, lt:lt + P],
                            pw_sb[:, i, k, :],
                            start=(k == 0),
                            stop=(k == 3),
                        )
                zh.append(zp)
            xt_f = xsb.tile([P, D_MODEL], dt.float32, tag="xf")
            x_dma.dma_start(xt_f[:, :], x_d[t0:t0 + P, :])
            z = zsb.tile([P, D_MODEL], dt.float32, tag="z", bufs=5)
            zs[tt] = z
            # z = zp + 64*x (whole chain is x64; exactly undone via the
            # x64-scaled EPS in the LN)
            for i in range(2):
                nc.vector.tensor_tensor(
                    z[:, i * D_HALF:(i + 1) * D_HALF],
                    zh[i][:, :],
                    xt_f[:, i * D_HALF:(i + 1) * D_HALF],
                    mybir.AluOpType.add,
                )
            # one-pass LN statistics on DVE (BN hardware): 2 subgroups of 512
            stats = lsb.tile([P, 2, 6], dt.float32, name="bnst", tag="bnst")
            for i in range(2):
                nc.vector.bn_stats(
                    out=stats[:, i, :],
                    in_=z[:, i * D_HALF:(i + 1) * D_HALF],
                )
            nc.vector.bn_aggr(out=mv4[:, tt, :], in_=stats[:, :, :])

        def _emit_p3b(s):
            # batched LN tail for the whole slot: one sqrt (one ACT table
            # swap pair per slot instead of per tile)
            gw = plan.regions[s]
            g0 = plan.offs[s]
            ntt = gw // P
            zs, mv4 = p3_state.pop(s)
            sig4 = lsb.tile([P, 4], dt.float32, name="sig4", tag="sig4", bufs=2)
            # unbiased sigma from biased bn variance: sqrt(var * N/(N-1))
            nc.scalar.activation(
                sig4[:, 0:ntt], mv4[:, 0:ntt, 1],
                mybir.ActivationFunctionType.Sqrt,
                scale=float(D_MODEL) / (D_MODEL - 1),
            )
            # the whole z chain is x64 (ident64 attention scale + x64
            # residual); scaling EPS by 64 keeps the LN output exact
            nc.gpsimd.tensor_scalar(
                sig4[:, 0:ntt], sig4[:, 0:ntt], ASCALE * EPS, None,
                mybir.AluOpType.add,
            )
            for tt in range(ntt):
                # scalars must be contiguous [P, 1] tiles: strided scalar
                # APs drop GpSimd's tensor_scalar into a ~15us slow path
                negmu = lsb.tile([P, 1], dt.float32, name="negmu",
                                 tag="negmu", bufs=2)
                nc.gpsimd.tensor_scalar(
                    negmu[:, :], mv4[:, tt, 0:1], -1.0, None,
                    mybir.AluOpType.mult,
                )
                rstd = lsb.tile([P, 1], dt.float32, name="rstd",
                                tag="rstd", bufs=2)
                nc.vector.reciprocal(rstd[:, :], sig4[:, tt:tt + 1])
                o = zsb.tile([P, D_MODEL], dt.float32, tag="o")
                eng_o = nc.gpsimd if ln_gp else nc.vector
                eng_o.tensor_scalar(
                    o[:, :], zs[tt][:, :], negmu[:, :], rstd[:, :],
                    mybir.AluOpType.add, mybir.AluOpType.mult,
                )
                out_dma.dma_start(out_d[g0 + tt * P:g0 + (tt + 1) * P, :],
                                  o[:, :])

        def _stage_prev(prev):
            # stage prev head's normalized (x64) attn^T slabs PSUM->SBUF as
            # fp8 DoubleRow chunk-pairs; odd tail chunk staged single
            ps, pot, pat, ph = prev
            L, gw, nk = _slot_dims(ps)
            # one staging copy for the whole head: [P, nk, L]
            ab = asb.tile([P, 4, 512], dt.float8e4, name="at_sb4",
                          tag="at_sb4", bufs=3)
            eng = nc.vector if ph % 2 else nc.scalar
            _copy(nc, eng, ab[:, 0:nk, 0:L], pat[:, 0:nk, 0:L])
            sbs = []
            if cfg.get("dr_av", True):
                for kp in range(nk // 2):
                    sbs.append((ab, True, 2 * kp))
                if nk % 2:
                    sbs.append((ab, False, nk - 1))
            else:
                for ki in range(nk):
                    sbs.append((ab, False, ki))
            return sbs

        def _prev_ot(prev, sbs, idx):
            # one attn@V chunk(-pair) of the prev head, streamed under the
            # current head's logits
            ps, pot, pat, ph = prev
            L, gw, nk = _slot_dims(ps)
            pvv = vv[ps].rearrange("p t (h d) -> p t h d", h=N_HEAD)
            ab, is_pair, k0 = sbs[idx]
            last = idx == len(sbs) - 1
            if is_pair:
                nc.tensor.matmul(
                    pot[:, 0:L],
                    pvv[0:P, k0:k0 + 2, ph, :],
                    ab[:, k0:k0 + 2, 0:L],
                    start=(idx == 0),
                    stop=last,
                    perf_mode=DR,
                )
            else:
                nc.tensor.matmul(
                    pot[:, 0:L],
                    pvv[0:P, k0, ph, :],
                    ab[:, k0, 0:L],
                    start=(idx == 0),
                    stop=last,
                )

        def _prev_out(prev):
            ps, pot, pat, ph = prev
            L, gw, nk = _slot_dims(ps)
            php, ppr = 64 * (ph % 2), ph // 2
            nc.vector.tensor_copy(
                o1t[ps][php:php + 64, ppr, 0:L], pot[0:64, 0:L])
            nc.vector.tensor_copy(
                o2t[ps][php:php + 64, ppr, 0:L], pot[64:128, 0:L])

        prev = None       # (slot, ot_psum, at_ps, head) not yet V-multiplied
        pend_p3 = None    # slot whose proj/LN is not yet emitted
        fill_q = []       # pending QKV emitters for the next slot
        for s in range(4):
            L, gw, nk = _slot_dims(s)
            nq = nk
            if s + 1 < 4:
                xt_next = _xt_load(xtp, s + 1)
                fill_q = _qkv_emitters(lgps, s + 1, xt_next)
            for h in range(N_HEAD):
                half, pr = h % 2, h // 2
                hp = 64 * half
                # attn^T slabs per key-chunk: [kc, all slot queries] so the
                # attn@V contraction runs one N=L matmul per chunk
                at_all = atps.tile([P, 4, 512], dt.float32, name="atp",
                                   tag="atp")
                at_ps = [at_all[:, ki, :] for ki in range(nk)]
                ot = ozps.tile([P, 512], dt.float32, name="ot", tag="ozp")
                sbs = _stage_prev(prev) if prev is not None else None
                nprev = len(sbs) if prev is not None else 0
                pend = {}
                for qi in range(nq):
                    qoff = P * qi
                    lq = min(P, L - P * qi)
                    lg = lgps.tile([P, 512], dt.float32, tag="lg")
                    # logits over the full padded region: pad keys are exact
                    # zeros -> exp contributes 1, corrected via npad
                    if cfg.get("dr_lg", True):
                        nc.tensor.matmul(
                            lg[0:lq, 0:gw],
                            qt[s][hp:hp + 64, :, pr, qoff:qoff + lq],
                            kt[s][hp:hp + 64, :, pr, 0:gw],
                            start=True,
                            stop=True,
                            perf_mode=DR,
                        )
                    else:
                        for comp in range(2):
                            nc.tensor.matmul(
                                lg[0:lq, 0:gw],
                                qt[s][hp:hp + 64, comp, pr, qoff:qoff + lq],
                                kt[s][hp:hp + 64, comp, pr, 0:gw],
                                start=(comp == 0),
                                stop=(comp == 1),
                            )
                    if prev is not None and qi < nprev:
                        _prev_ot(prev, sbs, qi)
                    # one next-slot QKV group per section: keeps the PE
                    # dense so the HAM clock never drops to 4/8
                    if fill_q:
                        fill_q.pop(0)()
                    attn = asb.tile([P, 512], dt.bfloat16, tag="attn")
                    se = ssb.tile([P, 1], dt.float32, tag="se")
                    # exp only over [0:L]; region-pad columns memset to zero
                    # so the transposes read exact zeros there
                    if L < gw:
                        nc.gpsimd.memset(attn[0:P, L:gw], 0.0)
                    nc.scalar.activation(
                        attn[0:lq, 0:L],
                        lg[0:lq, 0:L],
                        mybir.ActivationFunctionType.Exp,
                        scale=1.0 / SCALE,
                        accum_out=se[0:lq, :],
                    )
                    rc = ssb.tile([P, 1], dt.float32, tag="rc")
                    nc.gpsimd.tensor_tensor(
                        rc[0:lq, :], se[0:lq, :], npad_sb[0:lq, s:s + 1],
                        mybir.AluOpType.subtract,
                    )
                    nc.vector.reciprocal(rc[0:lq, :], rc[0:lq, :])
                    diag = ssb.tile([P, P], dt.bfloat16, tag="diag")
                    # ident is 64*I: diag = 64/denominator
                    nc.vector.tensor_scalar(
                        diag[0:lq, 0:lq], ident_sb[0:lq, 0:lq],
                        rc[0:lq, :], None, mybir.AluOpType.mult,
                    )
                    # transposes deferred two logits back: the
                    # exp->rc->diag chain gets ~2 sections of slack
                    if qi >= 2:
                        _transpose_qi(nc, at_ps, pend[qi - 2], nk)
                    pend[qi] = (attn, diag, lq, qoff)
                # flush: prev head's remaining ot chunks, last transposes
                if prev is not None:
                    for ki in range(nq, nprev):
                        _prev_ot(prev, sbs, ki)
                for qf in range(max(0, nq - 2), nq):
                    _transpose_qi(nc, at_ps, pend[qf], nk)
                if prev is not None:
                    _prev_out(prev)
                prev = (s, ot, at_all, h)
                if pend_p3 is not None and 3 <= h <= 6:
                    if h - 3 < plan.regions[pend_p3] // P:
                        _emit_p3a(pend_p3, h - 3)
                    if h == 6:
                        _emit_p3b(pend_p3)
                        pend_p3 = None
            # next slot's QKV must be complete before its attention starts
            while fill_q:
                fill_q.pop(0)()
            pend_p3 = s
        # drain the final head and the last slot's proj/LN
        sbs = _stage_prev(prev)
        for ki in range(len(sbs)):
            _prev_ot(prev, sbs, ki)
        _prev_out(prev)
        for tt in range(plan.regions[3] // P):
            _emit_p3a(3, tt)
        _emit_p3b(3)


_PROGRAMS = {}   # plan.key -> (nc, plan)
_RUNNERS = {}    # plan.key -> callable(in_maps) -> list[dict]


def _get_program(plan: Plan):
    if plan.key not in _PROGRAMS:
        _PROGRAMS[plan.key] = _build_program(plan)
    return _PROGRAMS[plan.key]


def _make_runner(nc, donate=True):
    """Cached PJRT runner (mirrors bass_utils.run_bass_kernel_spmd's axon
    path via bass2jax, but reuses the jitted executable across calls)."""
    import jax
    from jax.sharding import Mesh, PartitionSpec
    from jax.experimental.shard_map import shard_map
    from concourse import bass2jax

    bass2jax.install_neuronx_cc_hook()

    partition_name = (nc.partition_id_tensor.name
                      if nc.partition_id_tensor else None)
    in_names, out_names, out_avals, zero_shapes = [], [], [], []
    for alloc in nc.m.functions[0].allocations:
        if not isinstance(alloc, mybir.MemoryLocationSet):
            continue
        name = alloc.memorylocations[0].name
        if alloc.kind == "ExternalInput":
            if name == partition_name:
                continue
            in_names.append(name)
        elif alloc.kind == "ExternalOutput":
            out_names.append(name)
            shape = tuple(alloc.tensor_shape)
            dtype = mybir.dt.np(alloc.dtype)
            out_avals.append(jax.core.ShapedArray(shape, dtype))
            zero_shapes.append((shape, dtype))
    n_params = len(in_names)
    all_names = in_names + out_names
    if partition_name is not None:
        all_names = all_names + [partition_name]

    def _body(*args):
        operands = list(args)
        if partition_name is not None:
            operands.append(bass2jax.partition_id_tensor())
        outs = bass2jax._bass_exec_p.bind(
            *operands,
            out_avals=tuple(out_avals),
            in_names=tuple(all_names),
            out_names=tuple(out_names),
            lowering_input_output_aliases=(),
            sim_require_finite=True,
            sim_require_nnan=True,
            nc=nc,
        )
        return tuple(outs)

    devices = jax.devices()[:N_CORES]
    mesh = Mesh(np.asarray(devices), ("core",))
    in_specs = (PartitionSpec("core"),) * (n_params + len(out_names))
    out_specs = (PartitionSpec("core"),) * len(out_names)
    sharded = jax.jit(
        shard_map(_body, mesh=mesh, in_specs=in_specs, out_specs=out_specs,
                  check_rep=False),
        donate_argnums=tuple(range(n_params, n_params + len(out_names)))
        if donate else (),
        keep_unused=True,
    )

    def run(in_maps):
        concat_in = [
            np.concatenate([np.asarray(m[name]) for m in in_maps], axis=0)
            for name in in_names
        ]
        concat_zeros = [
            np.zeros((N_CORES * s[0], *s[1:]), d) for (s, d) in zero_shapes
        ]
        out_arrs = sharded(*concat_in, *concat_zeros)
        return [
            {
                name: np.asarray(out_arrs[i]).reshape(
                    N_CORES, *out_avals[i].shape)[c]
                for i, name in enumerate(out_names)
            }
            for c in range(N_CORES)
        ]

    run.sharded = sharded
    run.in_names = in_names
    run.out_names = out_names
    run.out_avals = out_avals
    run.zero_shapes = zero_shapes
    run.n_params = n_params
    return run


def _prep_weights(w_qs1, w_ks1, w_vs1, w_qs2, w_ks2, w_vs2, proj1_w, proj2_w):
    wq, wk, wv, pw = _prep_weights_4d(w_qs1, w_ks1, w_vs1, w_qs2, w_ks2,
                                      w_vs2, proj1_w, proj2_w)
    # partition-major packing: one contiguous DMA per weight tensor on device
    wq = np.ascontiguousarray(wq.transpose(2, 0, 1, 3).reshape(P, -1))
    wk = np.ascontiguousarray(wk.transpose(2, 0, 1, 3).reshape(P, -1))
    wv = np.ascontiguousarray(wv.transpose(1, 0, 2).reshape(P, -1))
    pw = np.ascontiguousarray(pw.transpose(2, 0, 1, 3).reshape(P, -1))
    return wq, wk, wv, pw


def _prep_weights_4d(w_qs1, w_ks1, w_vs1, w_qs2, w_ks2, w_vs2, proj1_w, proj2_w):
    wq = np.zeros((4, 8, P, P), FP8)
    wk = np.zeros((4, 8, P, P), FP8)
    for pr in range(4):
        h0, h1 = 2 * pr, 2 * pr + 1
        for j in range(8):
            if j < 4:
                rows = slice(j * P, (j + 1) * P)
                wq[pr, j] = np.concatenate(
                    [w_qs1[h0, rows, :], w_qs1[h1, rows, :]], axis=1).astype(FP8)
                wk[pr, j] = np.concatenate(
                    [w_ks1[h0, rows, :], w_ks1[h1, rows, :]], axis=1).astype(FP8)
            else:
                rows = slice((j - 4) * P, (j - 3) * P)
                wq[pr, j] = np.concatenate(
                    [w_qs2[h0, rows, :], w_qs2[h1, rows, :]], axis=1).astype(FP8)
                wk[pr, j] = np.concatenate(
                    [w_ks2[h0, rows, :], w_ks2[h1, rows, :]], axis=1).astype(FP8)
    wv = np.zeros((8, P, D_HALF), FP8)
    for j in range(8):
        src = w_vs1 if j < 4 else w_vs2
        rows = slice((j % 4) * P, (j % 4 + 1) * P)
        wv[j] = np.concatenate([src[h, rows, :] for h in range(8)], axis=1
                               ).astype(FP8)
    pw = np.zeros((2, 4, P, D_HALF), FP8)
    p1T = np.ascontiguousarray(proj1_w.T)  # [in, out]
    p2T = np.ascontiguousarray(proj2_w.T)
    for k in range(4):
        pw[0, k] = p1T[k * P:(k + 1) * P, :].astype(FP8)
        pw[1, k] = p2T[k * P:(k + 1) * P, :].astype(FP8)
    return wq, wk, wv, pw


def _prep_core_inputs(plan: Plan, inp, c):
    T = plan.t_pad
    x = np.zeros((T, D_MODEL), F32)
    npad = np.zeros((4,), F32)
    for j in range(4):
        s = plan.core_sents[c][j]
        L = int(plan.lengths[s])
        g0 = int(plan.glob_off[s])
        x[plan.offs[j]:plan.offs[j] + L] = inp[g0:g0 + L]
        # exp runs over [0, slot_pad); every pad key contributes
        # exp(0)=1 to the softmax denominator
        npad[j] = plan.slot_pad[j] - L
    # per-slot packed transpose: [p, c, t] = x[t, c*128+p], slots contiguous
    xT = np.zeros((P, 8 * T), FP8)
    for j in range(4):
        gw, g0 = plan.regions[j], plan.offs[j]
        blk = x[g0:g0 + gw].T.reshape(8, P, gw).transpose(1, 0, 2)
        xT[:, 8 * g0:8 * (g0 + gw)] = blk.reshape(P, 8 * gw).astype(FP8)
    npad_rep = np.tile(npad[None, :], (P, 1)).astype(F32)
    # residual ships pre-scaled x64 to match the x64 attention chain
    # (exact: power-of-two scale, undone via the x64-scaled LN epsilon)
    return x * ASCALE, xT, npad_rep


def make_in_maps(plan: Plan, inp, weights):
    wq, wk, wv, pw = weights
    ident = (np.eye(P) * ASCALE).astype(BF16)
    in_maps = []
    for c in range(N_CORES):
        x, xT, npad_rep = _prep_core_inputs(plan, inp, c)
        in_maps.append({
            "xT": xT, "x": x, "wq": wq, "wk": wk, "wv": wv, "pw": pw,
            "npad": npad_rep, "ident": ident,
        })
    return in_maps


def gather_output(plan: Plan, results, a_2=None, b_2=None):
    T_tot = int(plan.lengths.sum())
    out = np.empty((T_tot, D_MODEL), F32)
    for c in range(N_CORES):
        oc = results[c]["out"]
        for j in range(4):
            s = plan.core_sents[c][j]
            L = int(plan.lengths[s])
            g0 = int(plan.glob_off[s])
            out[g0:g0 + L] = oc[plan.offs[j]:plan.offs[j] + L]
    if a_2 is not None and (np.any(a_2 != 1.0) or np.any(b_2 != 0.0)):
        out = out * np.asarray(a_2, F32) + np.asarray(b_2, F32)
    return out


def kernel(inp, w_qs1, w_ks1, w_vs1, w_qs2, w_ks2, w_vs2,
           proj1_w, proj2_w, a_2, b_2, token_batch, token_pos, valid_mask):
    inp = np.asarray(inp, F32)
    token_batch = np.asarray(token_batch)
    lengths = np.bincount(token_batch, minlength=MB).astype(np.int64)
    # tokens of each sentence must be contiguous and in order
    plan = Plan(lengths)

    nc = _get_program(plan)
    if plan.key not in _RUNNERS:
        _RUNNERS[plan.key] = _make_runner(nc)
    runner = _RUNNERS[plan.key]

    weights = _prep_weights(np.asarray(w_qs1), np.asarray(w_ks1),
                            np.asarray(w_vs1), np.asarray(w_qs2),
                            np.asarray(w_ks2), np.asarray(w_vs2),
                            np.asarray(proj1_w), np.asarray(proj2_w))
    in_maps = make_in_maps(plan, inp, weights)
    results = runner(in_maps)
    return gather_output(plan, results, np.asarray(a_2), np.asarray(b_2))


# revision 37
# speedup vs baseline: 1.0198x; 1.0198x over previous
# Trainium2 Bass kernel for nn_MultiHeadAttention_24902220382931.
#
# Strategy: data-parallel over sentences. The 32 variable-length sentences are
# sorted by length; core c processes ranks {c, 15-c, 16+c, 31-c} (exactly equal
# token counts, near-equal attention work). Each core packs its 4 sentences
# into 4 fixed-size slots (max length per slot across cores, regions rounded to
# 128) so that all 8 cores execute one identical SPMD program. Padded rows are
# zeros; softmax denominators are corrected by subtracting the per-core pad
# count (pad keys contribute exp(0)=1 exactly), shipped as data.
#
# Precision: matmul operands in fp8e4 with DoubleRow perf mode (2x PE rate;
# contraction pairs packed as [K,2,N] APs), attn-transpose in bf16, softmax
# sum / residual / layernorm in fp32. Attention probabilities are scaled x64
# (to keep them in fp8's normal range) and descaled by 2^-6 at the fused
# residual add.
import sys

for _p in ("/opt/trn_rl_repo", "/root/.axon_site/_ro/trn_rl_repo"):
    if _p not in sys.path:
        sys.path.insert(0, _p)

import numpy as np
import ml_dtypes

import concourse.bass as bass  # noqa: F401  (bass types used via bacc/tile)
import concourse.mybir as mybir
import concourse.tile as tile
from concourse import bacc

BF16 = ml_dtypes.bfloat16
FP8 = ml_dtypes.float8_e4m3
F32 = np.float32

N_CORES = 8
MB = 32
D_MODEL = 1024
D_HALF = 512  # d_content == d_pos
N_HEAD = 8
D_K = 128
DK2 = 64
SCALE = float(D_MODEL) ** 0.5  # 32.0
EPS = 1e-3
P = 128  # partitions
ASCALE = 64.0     # attention-probability scale (fp8 normal range)
DESCALE = 1.0 / ASCALE

DR = mybir.MatmulPerfMode.DoubleRow


def _ceil_to(x, m):
    return (x + m - 1) // m * m


class Plan:
    def __init__(self, lengths):
        lengths = np.asarray(lengths, np.int64)
        assert lengths.shape == (MB,)
        order = np.argsort(-lengths, kind="stable")
        # core c handles sentence ranks {c, 15-c, 16+c, 31-c} (desc length order)
        self.core_sents = [
            [int(order[c]), int(order[15 - c]), int(order[16 + c]), int(order[31 - c])]
            for c in range(N_CORES)
        ]
        self.lengths = lengths
        self.slot_pad = [
            max(int(lengths[self.core_sents[c][j]]) for c in range(N_CORES))
            for j in range(4)
        ]
        self.regions = [_ceil_to(sp, P) for sp in self.slot_pad]
        self.offs = [0]
        for r in self.regions[:-1]:
            self.offs.append(self.offs[-1] + r)
        self.t_pad = sum(self.regions)
        assert self.t_pad % P == 0
        self.nt = self.t_pad // P
        self.glob_off = np.concatenate([[0], np.cumsum(lengths)[:-1]]).astype(np.int64)

    @property
    def key(self):
        return (tuple(self.slot_pad), self.t_pad)


def _copy(nc, eng, out, in_):
    # engine-dispatched copy: DVE has tensor_copy, ACT uses activation(Copy)
    if eng is nc.scalar:
        nc.scalar.copy(out, in_)
    else:
        eng.tensor_copy(out, in_)


def _transpose_qi(nc, at_ps, entry, nk):
    # attn^T @ diag(64*recip): transpose + normalize in one matmul per k-chunk
    attn, diag, lq, qoff = entry
    for ki in range(nk):
        nc.tensor.matmul(
            at_ps[ki][0:P, qoff:qoff + lq],
            attn[0:lq, P * ki:P * ki + P],
            diag[0:lq, 0:lq],
            start=True,
            stop=True,
        )


def _build_program(plan: Plan, loop_n: int = 1):
    """Build and compile the single-core Bass program (same for all cores).

    loop_n > 1 wraps the whole computation in a hardware For-loop (for
    steady-state timing measurements; the body is idempotent)."""
    import contextlib
    T = plan.t_pad
    nc = bacc.Bacc("TRN2", target_bir_lowering=False, debug=False)

    dt = mybir.dt
    # ---- DRAM I/O ----
    # xT packed per-slot: [p, s*(8*gw)] with [p, c, t] = x-dim c*128+p of
    # token t -> one contiguous descriptor per partition per slot
    xT_d = nc.dram_tensor("xT", [P, 8 * T], dt.float8e4, kind="ExternalInput").ap()
    x_d = nc.dram_tensor("x", [T, D_MODEL], dt.float32, kind="ExternalInput").ap()
    # weights packed partition-major on host: one contiguous DMA each
    wq_d = nc.dram_tensor("wq", [P, 4 * 8 * P], dt.float8e4, kind="ExternalInput").ap()
    wk_d = nc.dram_tensor("wk", [P, 4 * 8 * P], dt.float8e4, kind="ExternalInput").ap()
    wv_d = nc.dram_tensor("wv", [P, 8 * D_HALF], dt.float8e4, kind="ExternalInput").ap()
    pw_d = nc.dram_tensor("pw", [P, 8 * D_HALF], dt.float8e4, kind="ExternalInput").ap()
    npad_d = nc.dram_tensor("npad", [P, 4], dt.float32, kind="ExternalInput").ap()
    ident_d = nc.dram_tensor("ident", [P, P], dt.bfloat16, kind="ExternalInput").ap()
    out_d = nc.dram_tensor("out", [T, D_MODEL], dt.float32, kind="ExternalOutput").ap()

    with tile.TileContext(nc) as tc:
        with (
            tc.tile_pool(name="persist", bufs=1) as pp,
            tc.tile_pool(name="weights", bufs=1) as wp,
        ):
            # Per-slot persistent tensors (finer dependency granularity lets
            # attention/proj start as soon as a slot's QKV is done).
            # Q^T/K^T pair-stacked: [p, comp(c/p), pair, region]; partition
            # p<64 holds head 2*pair, p>=64 head 2*pair+1 (comp's 64 dims).
            qt = [pp.tile([P, 2, 4, r], dt.float8e4, name=f"qt{s}", tag=f"qt{s}")
                  for s, r in enumerate(plan.regions)]
            kt = [pp.tile([P, 2, 4, r], dt.float8e4, name=f"kt{s}", tag=f"kt{s}")
                  for s, r in enumerate(plan.regions)]
            # V token-natural, head-major columns: [p, tile, head, {c64|p64}]
            vv = [pp.tile([P, r // P, D_MODEL], dt.float8e4, name=f"vv{s}", tag=f"vv{s}")
                  for s, r in enumerate(plan.regions)]
            o1t = [pp.tile([P, 4, r], dt.float8e4, name=f"o1t{s}", tag=f"o1t{s}")
                   for s, r in enumerate(plan.regions)]
            o2t = [pp.tile([P, 4, r], dt.float8e4, name=f"o2t{s}", tag=f"o2t{s}")
                   for s, r in enumerate(plan.regions)]
            npad_sb = pp.tile([P, 4], dt.float32, tag="npad")
            ident_sb = pp.tile([P, P], dt.bfloat16, tag="ident")

            wq_sb = wp.tile([P, 4, 8, P], dt.float8e4, tag="wq")
            wk_sb = wp.tile([P, 4, 8, P], dt.float8e4, tag="wk")
            wv_sb = wp.tile([P, 8, D_HALF], dt.float8e4, tag="wv")
            pw_sb = wp.tile([P, 2, 4, D_HALF], dt.float8e4, tag="pw")

            _c = getattr(plan, "cfg", {})
            aux_eng = getattr(nc, _c.get("aux_eng", "gpsimd"))
            w_eng = getattr(nc, _c.get("w_eng", "scalar"))
            aux_eng.dma_start(npad_sb[:, :], npad_d[:, :])
            aux_eng.dma_start(ident_sb[:, :], ident_d[:, :])
            # per-pr-chunk weight DMAs on separate queues: the first QK
            # matmuls are gated on a small chunk, not the full weight set
            wq_f = wq_sb.rearrange("p a b c -> p a (b c)")
            wk_f = wk_sb.rearrange("p a b c -> p a (b c)")
            wq_r = wq_d.rearrange("p (a r) -> p a r", a=4)
            wk_r = wk_d.rearrange("p (a r) -> p a r", a=4)
            for pr in range(4):
                w_eng.dma_start(wq_f[:, pr, :], wq_r[:, pr, :])
                w_eng.dma_start(wk_f[:, pr, :], wk_r[:, pr, :])
            aux_eng.dma_start(
                wv_sb.rearrange("p a b -> p (a b)")[:, :], wv_d[:, :])
            aux_eng.dma_start(
                pw_sb.rearrange("p a b c -> p (a b c)")[:, :], pw_d[:, :])

            loop_cm = (tc.For_i(0, loop_n, 1,
                                hint_engines=(mybir.EngineType.PE,
                                              mybir.EngineType.DVE,
                                              mybir.EngineType.Activation,
                                              mybir.EngineType.SP))
                       if loop_n > 1 else contextlib.nullcontext())
            with loop_cm:
                _kernel_body(nc, tc, plan, locals())

    nc.compile()
    return nc


def _kernel_body(nc, tc, plan, env):
    dt = mybir.dt
    qt, kt, vv, o1t, o2t = (env["qt"], env["kt"], env["vv"], env["o1t"],
                            env["o2t"])
    npad_sb, ident_sb = env["npad_sb"], env["ident_sb"]
    wq_sb, wk_sb, wv_sb, pw_sb = (env["wq_sb"], env["wk_sb"], env["wv_sb"],
                                  env["pw_sb"])
    xT_d, x_d, out_d = env["xT_d"], env["x_d"], env["out_d"]
    # dr_*: which matmul groups use fp8 DoubleRow. The board power limiter
    # halves the PE clock under sustained full-DR load, so only the QKV
    # projections (+ the half-array logits, which are power-neutral) and the
    # proj run DoubleRow; attn@V stays plain-fp8. Empirically fastest mix.
    cfg = dict(qk=4, v=3, lg=2, at=1, ot=1, z=6,
               dr_qkv=True, dr_lg=True, dr_av=True, dr_proj=False)
    cfg.update(getattr(plan, "cfg", {}))
    out_eng = getattr(nc, cfg.get("out_eng", "gpsimd"))
    aux_eng = getattr(nc, cfg.get("aux_eng", "gpsimd"))
    ms_gp = cfg.get("ms_gp", True)
    ln_gp = cfg.get("ln_gp", True)

    # zero the attention-output staging (pad-query columns are never
    # written; keep them finite for the projection matmuls)
    ms_eng = nc.gpsimd if ms_gp else nc.vector
    for s in range(4):
        L, r = plan.slot_pad[s], plan.regions[s]
        if L < r:
            ms_eng.memset(o1t[s][:, :, L:r], 0.0)
            ms_eng.memset(o2t[s][:, :, L:r], 0.0)

    # ============ Phase 1: QKV projection emitters ============
    # Only slot 0 runs as a dense up-front phase; slots 1-3's QKV matmuls
    # are interleaved as PE filler into the previous slot's attention so
    # the PE never idles long enough for the HAM to re-throttle its clock.
    dr_qkv = cfg.get("dr_qkv", True)

    def _xt_load(xtp, s):
        gw = plan.regions[s]
        g0 = plan.offs[s]
        xt_sb = xtp.tile([P, 8, gw], dt.float8e4, name="xt", tag="xt")
        xt_f2 = xt_sb.rearrange("p a b -> p (a b)")
        # split across several DMA queues (1-1.5KB/partition descriptors)
        for c in range(4):
            nc.sync.dma_start(
                xt_f2[:, 2 * c * gw:2 * (c + 1) * gw],
                xT_d[:, 8 * g0 + 2 * c * gw:8 * g0 + 2 * (c + 1) * gw])
        return xt_sb

    def _qkv_emitters(pool, s, xt_sb, pr_major=False):
        """Closures, each emitting one PSUM acc group (matmuls + copy)."""
        gw = plan.regions[s]
        ems = []

        def _qk_group(half, pr, qk):
            w_sb, dst = ((wq_sb, qt), (wk_sb, kt))[qk]
            acc = pool.tile([P, 512], dt.float32, name="qkacc", tag="lg")
            if dr_qkv:
                for jj in range(2):
                    j0 = half * 4 + 2 * jj
                    nc.tensor.matmul(
                        acc[:, 0:gw],
                        w_sb[:, pr, j0:j0 + 2, :],
                        xt_sb[:, j0:j0 + 2, 0:gw],
                        start=(jj == 0),
                        stop=(jj == 1),
                        perf_mode=DR,
                    )
            else:
                for jj in range(4):
                    j = half * 4 + jj
                    nc.tensor.matmul(
                        acc[:, 0:gw],
                        w_sb[:, pr, j, :],
                        xt_sb[:, j, 0:gw],
                        start=(jj == 0),
                        stop=(jj == 3),
                    )
            eng = nc.vector if (pr + qk + half) % 2 else nc.scalar
            _copy(nc, eng, dst[s][:, half, pr, 0:gw], acc[:, 0:gw])

        vv_w = vv[s].rearrange("p t (h b d) -> p t h b d", h=N_HEAD, b=2)

        def _v_group(tt, half):
            tl = tt * P
            vacc = pool.tile([P, 512], dt.float32, name="vacc", tag="lg")
            vacc_r = vacc.rearrange("p (h d) -> p h d", h=N_HEAD)
            if dr_qkv:
                for jj in range(2):
                    j0 = half * 4 + 2 * jj
                    nc.tensor.matmul(
                        vacc[:, :],
                        xt_sb[:, j0:j0 + 2, tl:tl + P],
                        wv_sb[:, j0:j0 + 2, :],
                        start=(jj == 0),
                        stop=(jj == 1),
                        perf_mode=DR,
                    )
            else:
                for jj in range(4):
                    j = half * 4 + jj
                    nc.tensor.matmul(
                        vacc[:, :],
                        xt_sb[:, j, tl:tl + P],
                        wv_sb[:, j, :],
                        start=(jj == 0),
                        stop=(jj == 3),
                    )
            nc.scalar.copy(vv_w[:, tt, :, half, :], vacc_r[:, :, :])

        if pr_major:
            # up-front slot: pr-major order so head 0's q/k (pr=0, both
            # halves) complete first and attention starts earliest
            for pr in range(4):
                for half in range(2):
                    for qk in range(2):
                        ems.append(lambda h=half, p=pr, q=qk: _qk_group(h, p, q))
            for half in range(2):
                for tt in range(gw // P):
                    ems.append(lambda t=tt, h=half: _v_group(t, h))
        else:
            # half-0 groups first: PE can start while xt half-1 is landing
            for half in range(2):
                for pr in range(4):
                    for qk in range(2):
                        ems.append(lambda h=half, p=pr, q=qk: _qk_group(h, p, q))
                for tt in range(gw // P):
                    ems.append(lambda t=tt, h=half: _v_group(t, h))
        return ems

    # dense up-front QKV for slot 0 in its own (wider) PSUM scope
    with (
        tc.tile_pool(name="xt0_pool", bufs=1) as xtp0,
        tc.tile_pool(name="p1_ps", bufs=7, space="PSUM") as p1ps,
    ):
        xt0 = _xt_load(xtp0, 0)
        for em in _qkv_emitters(p1ps, 0, xt0, pr_major=True):
            em()

    # ======== Phase 2+3: attention + proj/LN + QKV filler ========
    # PSUM: lg/filler(2) + atp0..3(4) + ozp(2, attn-out + proj acc) = 8
    x_dma = nc.sync
    out_dma = out_eng
    with (
        tc.tile_pool(name="xt_pool", bufs=2) as xtp,
        tc.tile_pool(name="lg_ps", bufs=2, space="PSUM") as lgps,
        tc.tile_pool(name="at_ps", bufs=1, space="PSUM") as atps,
        tc.tile_pool(name="ozp_ps", bufs=2, space="PSUM") as ozps,
        tc.tile_pool(name="attn_sb", bufs=6) as asb,
        tc.tile_pool(name="small_sb", bufs=8) as ssb,
        tc.tile_pool(name="z_sb", bufs=3) as zsb,
        tc.tile_pool(name="x_sb", bufs=6) as xsb,
        tc.tile_pool(name="ln_sb", bufs=4) as lsb,
    ):
        def _slot_dims(s):
            gw = plan.regions[s]
            L = plan.slot_pad[s]
            nk = gw // P
            return L, gw, nk

        p3_state = {}  # slot -> (z tiles, mv4 tile)

        def _emit_p3a(s, tt):
            # proj + residual + one-pass LN stats for one token tile
            g0 = plan.offs[s]
            lt = tt * P
            t0 = g0 + lt
            if tt == 0:
                ntt0 = plan.regions[s] // P
                xfs = []
                for t2 in range(ntt0):
                    xf = xsb.tile([P, D_MODEL], dt.float32, name="xf",
                                  tag="xf", bufs=6)
                    x_dma.dma_start(xf[:, :], x_d[g0 + t2 * P:g0 + (t2 + 1) * P, :])
                    xfs.append(xf)
                p3_state[s] = ([None] * 4,
                               lsb.tile([P, 4, 2], dt.float32, name="mv4", tag="mv4",
                                        bufs=2),
                               xfs)
            zs, mv4, xfs = p3_state[s]
            zh = []
            for i, osrc in enumerate((o1t[s], o2t[s])):
                zp = ozps.tile([P, 512], dt.float32, name="zp", tag="ozp")
                if cfg.get("dr_proj", True):
                    for kp in range(2):
                        nc.tensor.matmul(
                            zp[:, :],
                            osrc[:, 2 * kp:2 * kp + 2, lt:lt + P],
                            pw_sb[:, i, 2 * kp:2 * kp + 2, :],
                            start=(kp == 0),
                            stop=(kp == 1),
                            perf_mode=DR,
                        )
                else:
                    for k in range(4):
                        nc.tensor.matmul(
                            zp[:, :],
                            osrc[:, k, lt:lt + P],
                            pw_sb[:, i, k, :],
                            start=(k == 0),
                            stop=(k == 3),
                        )
                zh.append(zp)
            xt_f = xfs[tt]
            z = zsb.tile([P, D_MODEL], dt.float32, tag="z", bufs=5)
            zs[tt] = z
            # z = zp + 64*x (whole chain is x64; exactly undone via the
            # x64-scaled EPS in the LN)
            for i in range(2):
                nc.vector.tensor_tensor(
                    z[:, i * D_HALF:(i + 1) * D_HALF],
                    zh[i][:, :],
                    xt_f[:, i * D_HALF:(i + 1) * D_HALF],
                    mybir.AluOpType.add,
                )
            # one-pass LN statistics on DVE (BN hardware): 2 subgroups of 512
            stats = lsb.tile([P, 2, 6], dt.float32, name="bnst", tag="bnst")
            for i in range(2):
                nc.vector.bn_stats(
                    out=stats[:, i, :],
                    in_=z[:, i * D_HALF:(i + 1) * D_HALF],
                )
            nc.vector.bn_aggr(out=mv4[:, tt, :], in_=stats[:, :, :])

        def _emit_p3b(s):
            # batched LN tail for the whole slot: one sqrt (one ACT table
            # swap pair per slot instead of per tile)
            gw = plan.regions[s]
            g0 = plan.offs[s]
            ntt = gw // P
            zs, mv4, _ = p3_state.pop(s)
            sig4 = lsb.tile([P, 4], dt.float32, name="sig4", tag="sig4", bufs=2)
            # unbiased sigma from biased bn variance: sqrt(var * N/(N-1))
            nc.scalar.activation(
                sig4[:, 0:ntt], mv4[:, 0:ntt, 1],
                mybir.ActivationFunctionType.Sqrt,
                scale=float(D_MODEL) / (D_MODEL - 1),
            )
            # the whole z chain is x64 (ident64 attention scale + x64
            # residual); scaling EPS by 64 keeps the LN output exact
            nc.gpsimd.tensor_scalar(
                sig4[:, 0:ntt], sig4[:, 0:ntt], ASCALE * EPS, None,
                mybir.AluOpType.add,
            )
            for tt in range(ntt):
                # scalars must be contiguous [P, 1] tiles: strided scalar
                # APs drop GpSimd's tensor_scalar into a ~15us slow path
                negmu = lsb.tile([P, 1], dt.float32, name="negmu",
                                 tag="negmu", bufs=2)
                nc.gpsimd.tensor_scalar(
                    negmu[:, :], mv4[:, tt, 0:1], -1.0, None,
                    mybir.AluOpType.mult,
                )
                rstd = lsb.tile([P, 1], dt.float32, name="rstd",
                                tag="rstd", bufs=2)
                nc.vector.reciprocal(rstd[:, :], sig4[:, tt:tt + 1])
                o = zsb.tile([P, D_MODEL], dt.float32, tag="o")
                eng_o = nc.gpsimd if ln_gp else nc.vector
                eng_o.tensor_scalar(
                    o[:, :], zs[tt][:, :], negmu[:, :], rstd[:, :],
                    mybir.AluOpType.add, mybir.AluOpType.mult,
                )
                out_dma.dma_start(out_d[g0 + tt * P:g0 + (tt + 1) * P, :],
                                  o[:, :])

        def _stage_prev(prev):
            # stage prev head's normalized (x64) attn^T slabs PSUM->SBUF as
            # fp8 DoubleRow chunk-pairs; odd tail chunk staged single
            ps, pot, pat, ph = prev
            L, gw, nk = _slot_dims(ps)
            # one staging copy for the whole head: [P, nk, L]
            ab = asb.tile([P, 4, 512], dt.float8e4, name="at_sb4",
                          tag="at_sb4", bufs=3)
            eng = nc.vector if ph % 2 else nc.scalar
            _copy(nc, eng, ab[:, 0:nk, 0:L], pat[:, 0:nk, 0:L])
            sbs = []
            if cfg.get("dr_av", True):
                for kp in range(nk // 2):
                    sbs.append((ab, True, 2 * kp))
                if nk % 2:
                    sbs.append((ab, False, nk - 1))
            else:
                for ki in range(nk):
                    sbs.append((ab, False, ki))
            return sbs

        def _prev_ot(prev, sbs, idx):
            # one attn@V chunk(-pair) of the prev head, streamed under the
            # current head's logits
            ps, pot, pat, ph = prev
            L, gw, nk = _slot_dims(ps)
            pvv = vv[ps].rearrange("p t (h d) -> p t h d", h=N_HEAD)
            ab, is_pair, k0 = sbs[idx]
            last = idx == len(sbs) - 1
            if is_pair:
                nc.tensor.matmul(
                    pot[:, 0:L],
                    pvv[0:P, k0:k0 + 2, ph, :],
                    ab[:, k0:k0 + 2, 0:L],
                    start=(idx == 0),
                    stop=last,
                    perf_mode=DR,
                )
            else:
                nc.tensor.matmul(
                    pot[:, 0:L],
                    pvv[0:P, k0, ph, :],
                    ab[:, k0, 0:L],
                    start=(idx == 0),
                    stop=last,
                )

        def _prev_out(prev):
            ps, pot, pat, ph = prev
            L, gw, nk = _slot_dims(ps)
            php, ppr = 64 * (ph % 2), ph // 2
            nc.vector.tensor_copy(
                o1t[ps][php:php + 64, ppr, 0:L], pot[0:64, 0:L])
            nc.vector.tensor_copy(
                o2t[ps][php:php + 64, ppr, 0:L], pot[64:128, 0:L])

        prev = None       # (slot, ot_psum, at_ps, head) not yet V-multiplied
        pend_p3 = None    # slot whose proj/LN is not yet emitted
        fill_q = []       # pending QKV emitters for the next slot
        for s in range(4):
            L, gw, nk = _slot_dims(s)
            nq = nk
            if s + 1 < 4:
                xt_next = _xt_load(xtp, s + 1)
                fill_q = _qkv_emitters(lgps, s + 1, xt_next)
            for h in range(N_HEAD):
                half, pr = h % 2, h // 2
                hp = 64 * half
                # attn^T slabs per key-chunk: [kc, all slot queries] so the
                # attn@V contraction runs one N=L matmul per chunk
                at_all = atps.tile([P, 4, 512], dt.float32, name="atp",
                                   tag="atp")
                at_ps = [at_all[:, ki, :] for ki in range(nk)]
                ot = ozps.tile([P, 512], dt.float32, name="ot", tag="ozp")
                sbs = _stage_prev(prev) if prev is not None else None
                nprev = len(sbs) if prev is not None else 0
                pend = {}
                for qi in range(nq):
                    qoff = P * qi
                    lq = min(P, L - P * qi)
                    lg = lgps.tile([P, 512], dt.float32, tag="lg")
                    # logits over the full padded region: pad keys are exact
                    # zeros -> exp contributes 1, corrected via npad
                    if cfg.get("dr_lg", True):
                        nc.tensor.matmul(
                            lg[0:lq, 0:gw],
                            qt[s][hp:hp + 64, :, pr, qoff:qoff + lq],
                            kt[s][hp:hp + 64, :, pr, 0:gw],
                            start=True,
                            stop=True,
                            perf_mode=DR,
                        )
                    else:
                        for comp in range(2):
                            nc.tensor.matmul(
                                lg[0:lq, 0:gw],
                                qt[s][hp:hp + 64, comp, pr, qoff:qoff + lq],
                                kt[s][hp:hp + 64, comp, pr, 0:gw],
                                start=(comp == 0),
                                stop=(comp == 1),
                            )
                    if prev is not None and qi < nprev:
                        _prev_ot(prev, sbs, qi)
                    # one next-slot QKV group per section: keeps the PE
                    # dense so the HAM clock never drops to 4/8
                    if fill_q:
                        fill_q.pop(0)()
                    attn = asb.tile([P, 512], dt.bfloat16, tag="attn")
                    se = ssb.tile([P, 1], dt.float32, tag="se")
                    # exp only over [0:L]; region-pad columns memset to zero
                    # so the transposes read exact zeros there
                    if L < gw:
                        nc.gpsimd.memset(attn[0:P, L:gw], 0.0)
                    nc.scalar.activation(
                        attn[0:lq, 0:L],
                        lg[0:lq, 0:L],
                        mybir.ActivationFunctionType.Exp,
                        scale=1.0 / SCALE,
                        accum_out=se[0:lq, :],
                    )
                    rc = ssb.tile([P, 1], dt.float32, tag="rc")
                    nc.gpsimd.tensor_tensor(
                        rc[0:lq, :], se[0:lq, :], npad_sb[0:lq, s:s + 1],
                        mybir.AluOpType.subtract,
                    )
                    nc.vector.reciprocal(rc[0:lq, :], rc[0:lq, :])
                    diag = ssb.tile([P, P], dt.bfloat16, tag="diag")
                    # ident is 64*I: diag = 64/denominator
                    nc.vector.tensor_scalar(
                        diag[0:lq, 0:lq], ident_sb[0:lq, 0:lq],
                        rc[0:lq, :], None, mybir.AluOpType.mult,
                    )
                    # transposes deferred two logits back: the
                    # exp->rc->diag chain gets ~2 sections of slack
                    if qi >= 2:
                        _transpose_qi(nc, at_ps, pend[qi - 2], nk)
                    pend[qi] = (attn, diag, lq, qoff)
                # flush: prev head's remaining ot chunks, last transposes
                if prev is not None:
                    for ki in range(nq, nprev):
                        _prev_ot(prev, sbs, ki)
                for qf in range(max(0, nq - 2), nq):
                    _transpose_qi(nc, at_ps, pend[qf], nk)
                if prev is not None:
                    _prev_out(prev)
                prev = (s, ot, at_all, h)
                if pend_p3 is not None and 3 <= h <= 6:
                    if h - 3 < plan.regions[pend_p3] // P:
                        _emit_p3a(pend_p3, h - 3)
                    if h == 6:
                        _emit_p3b(pend_p3)
                        pend_p3 = None
            # next slot's QKV must be complete before its attention starts
            while fill_q:
                fill_q.pop(0)()
            pend_p3 = s
        # drain the final head and the last slot's proj/LN
        sbs = _stage_prev(prev)
        for ki in range(len(sbs)):
            _prev_ot(prev, sbs, ki)
        _prev_out(prev)
        for tt in range(plan.regions[3] // P):
            _emit_p3a(3, tt)
        _emit_p3b(3)


_PROGRAMS = {}   # plan.key -> (nc, plan)
_RUNNERS = {}    # plan.key -> callable(in_maps) -> list[dict]


def _get_program(plan: Plan):
    if plan.key not in _PROGRAMS:
        _PROGRAMS[plan.key] = _build_program(plan)
    return _PROGRAMS[plan.key]


def _make_runner(nc, donate=True):
    """Cached PJRT runner (mirrors bass_utils.run_bass_kernel_spmd's axon
    path via bass2jax, but reuses the jitted executable across calls)."""
    import jax
    from jax.sharding import Mesh, PartitionSpec
    from jax.experimental.shard_map import shard_map
    from concourse import bass2jax

    bass2jax.install_neuronx_cc_hook()

    partition_name = (nc.partition_id_tensor.name
                      if nc.partition_id_tensor else None)
    in_names, out_names, out_avals, zero_shapes = [], [], [], []
    for alloc in nc.m.functions[0].allocations:
        if not isinstance(alloc, mybir.MemoryLocationSet):
            continue
        name = alloc.memorylocations[0].name
        if alloc.kind == "ExternalInput":
            if name == partition_name:
                continue
            in_names.append(name)
        elif alloc.kind == "ExternalOutput":
            out_names.append(name)
            shape = tuple(alloc.tensor_shape)
            dtype = mybir.dt.np(alloc.dtype)
            out_avals.append(jax.core.ShapedArray(shape, dtype))
            zero_shapes.append((shape, dtype))
    n_params = len(in_names)
    all_names = in_names + out_names
    if partition_name is not None:
        all_names = all_names + [partition_name]

    def _body(*args):
        operands = list(args)
        if partition_name is not None:
            operands.append(bass2jax.partition_id_tensor())
        outs = bass2jax._bass_exec_p.bind(
            *operands,
            out_avals=tuple(out_avals),
            in_names=tuple(all_names),
            out_names=tuple(out_names),
            lowering_input_output_aliases=(),
            sim_require_finite=True,
            sim_require_nnan=True,
            nc=nc,
        )
        return tuple(outs)

    devices = jax.devices()[:N_CORES]
    mesh = Mesh(np.asarray(devices), ("core",))
    in_specs = (PartitionSpec("core"),) * (n_params + len(out_names))
    out_specs = (PartitionSpec("core"),) * len(out_names)
    sharded = jax.jit(
        shard_map(_body, mesh=mesh, in_specs=in_specs, out_specs=out_specs,
                  check_rep=False),
        donate_argnums=tuple(range(n_params, n_params + len(out_names)))
        if donate else (),
        keep_unused=True,
    )

    def run(in_maps):
        concat_in = [
            np.concatenate([np.asarray(m[name]) for m in in_maps], axis=0)
            for name in in_names
        ]
        concat_zeros = [
            np.zeros((N_CORES * s[0], *s[1:]), d) for (s, d) in zero_shapes
        ]
        out_arrs = sharded(*concat_in, *concat_zeros)
        return [
            {
                name: np.asarray(out_arrs[i]).reshape(
                    N_CORES, *out_avals[i].shape)[c]
                for i, name in enumerate(out_names)
            }
            for c in range(N_CORES)
        ]

    run.sharded = sharded
    run.in_names = in_names
    run.out_names = out_names
    run.out_avals = out_avals
    run.zero_shapes = zero_shapes
    run.n_params = n_params
    return run


def _prep_weights(w_qs1, w_ks1, w_vs1, w_qs2, w_ks2, w_vs2, proj1_w, proj2_w):
    wq, wk, wv, pw = _prep_weights_4d(w_qs1, w_ks1, w_vs1, w_qs2, w_ks2,
                                      w_vs2, proj1_w, proj2_w)
    # partition-major packing: one contiguous DMA per weight tensor on device
    wq = np.ascontiguousarray(wq.transpose(2, 0, 1, 3).reshape(P, -1))
    wk = np.ascontiguousarray(wk.transpose(2, 0, 1, 3).reshape(P, -1))
    wv = np.ascontiguousarray(wv.transpose(1, 0, 2).reshape(P, -1))
    pw = np.ascontiguousarray(pw.transpose(2, 0, 1, 3).reshape(P, -1))
    return wq, wk, wv, pw


def _prep_weights_4d(w_qs1, w_ks1, w_vs1, w_qs2, w_ks2, w_vs2, proj1_w, proj2_w):
    wq = np.zeros((4, 8, P, P), FP8)
    wk = np.zeros((4, 8, P, P), FP8)
    for pr in range(4):
        h0, h1 = 2 * pr, 2 * pr + 1
        for j in range(8):
            if j < 4:
                rows = slice(j * P, (j + 1) * P)
                wq[pr, j] = np.concatenate(
                    [w_qs1[h0, rows, :], w_qs1[h1, rows, :]], axis=1).astype(FP8)
                wk[pr, j] = np.concatenate(
                    [w_ks1[h0, rows, :], w_ks1[h1, rows, :]], axis=1).astype(FP8)
            else:
                rows = slice((j - 4) * P, (j - 3) * P)
                wq[pr, j] = np.concatenate(
                    [w_qs2[h0, rows, :], w_qs2[h1, rows, :]], axis=1).astype(FP8)
                wk[pr, j] = np.concatenate(
                    [w_ks2[h0, rows, :], w_ks2[h1, rows, :]], axis=1).astype(FP8)
    wv = np.zeros((8, P, D_HALF), FP8)
    for j in range(8):
        src = w_vs1 if j < 4 else w_vs2
        rows = slice((j % 4) * P, (j % 4 + 1) * P)
        wv[j] = np.concatenate([src[h, rows, :] for h in range(8)], axis=1
                               ).astype(FP8)
    pw = np.zeros((2, 4, P, D_HALF), FP8)
    p1T = np.ascontiguousarray(proj1_w.T)  # [in, out]
    p2T = np.ascontiguousarray(proj2_w.T)
    for k in range(4):
        pw[0, k] = p1T[k * P:(k + 1) * P, :].astype(FP8)
        pw[1, k] = p2T[k * P:(k + 1) * P, :].astype(FP8)
    return wq, wk, wv, pw


def _prep_core_inputs(plan: Plan, inp, c):
    T = plan.t_pad
    x = np.zeros((T, D_MODEL), F32)
    npad = np.zeros((4,), F32)
    for j in range(4):
        s = plan.core_sents[c][j]
        L = int(plan.lengths[s])
        g0 = int(plan.glob_off[s])
        x[plan.offs[j]:plan.offs[j] + L] = inp[g0:g0 + L]
        # exp runs over [0, slot_pad); every pad key contributes
        # exp(0)=1 to the softmax denominator
        npad[j] = plan.slot_pad[j] - L
    # per-slot packed transpose: [p, c, t] = x[t, c*128+p], slots contiguous
    xT = np.zeros((P, 8 * T), FP8)
    for j in range(4):
        gw, g0 = plan.regions[j], plan.offs[j]
        blk = x[g0:g0 + gw].T.reshape(8, P, gw).transpose(1, 0, 2)
        xT[:, 8 * g0:8 * (g0 + gw)] = blk.reshape(P, 8 * gw).astype(FP8)
    npad_rep = np.tile(npad[None, :], (P, 1)).astype(F32)
    # residual ships pre-scaled x64 to match the x64 attention chain
    # (exact: power-of-two scale, undone via the x64-scaled LN epsilon)
    return x * ASCALE, xT, npad_rep


def make_in_maps(plan: Plan, inp, weights):
    wq, wk, wv, pw = weights
    ident = (np.eye(P) * ASCALE).astype(BF16)
    in_maps = []
    for c in range(N_CORES):
        x, xT, npad_rep = _prep_core_inputs(plan, inp, c)
        in_maps.append({
            "xT": xT, "x": x, "wq": wq, "wk": wk, "wv": wv, "pw": pw,
            "npad": npad_rep, "ident": ident,
        })
    return in_maps


def gather_output(plan: Plan, results, a_2=None, b_2=None):
    T_tot = int(plan.lengths.sum())
    out = np.empty((T_tot, D_MODEL), F32)
    for c in range(N_CORES):
        oc = results[c]["out"]
        for j in range(4):
            s = plan.core_sents[c][j]
            L = int(plan.lengths[s])
            g0 = int(plan.glob_off[s])
            out[g0:g0 + L] = oc[plan.offs[j]:plan.offs[j] + L]
    if a_2 is not None and (np.any(a_2 != 1.0) or np.any(b_2 != 0.0)):
        out = out * np.asarray(a_2, F32) + np.asarray(b_2, F32)
    return out


def kernel(inp, w_qs1, w_ks1, w_vs1, w_qs2, w_ks2, w_vs2,
           proj1_w, proj2_w, a_2, b_2, token_batch, token_pos, valid_mask):
    inp = np.asarray(inp, F32)
    token_batch = np.asarray(token_batch)
    lengths = np.bincount(token_batch, minlength=MB).astype(np.int64)
    # tokens of each sentence must be contiguous and in order
    plan = Plan(lengths)

    nc = _get_program(plan)
    if plan.key not in _RUNNERS:
        _RUNNERS[plan.key] = _make_runner(nc)
    runner = _RUNNERS[plan.key]

    weights = _prep_weights(np.asarray(w_qs1), np.asarray(w_ks1),
                            np.asarray(w_vs1), np.asarray(w_qs2),
                            np.asarray(w_ks2), np.asarray(w_vs2),
                            np.asarray(proj1_w), np.asarray(proj2_w))
    in_maps = make_in_maps(plan, inp, weights)
    results = runner(in_maps)
    return gather_output(plan, results, np.asarray(a_2), np.asarray(b_2))


# revision 39
# speedup vs baseline: 1.0250x; 1.0051x over previous
# Trainium2 Bass kernel for nn_MultiHeadAttention_24902220382931.
#
# Strategy: data-parallel over sentences. The 32 variable-length sentences are
# sorted by length; core c processes ranks {c, 15-c, 16+c, 31-c} (exactly equal
# token counts, near-equal attention work). Each core packs its 4 sentences
# into 4 fixed-size slots (max length per slot across cores, regions rounded to
# 128) so that all 8 cores execute one identical SPMD program. Padded rows are
# zeros; softmax denominators are corrected by subtracting the per-core pad
# count (pad keys contribute exp(0)=1 exactly), shipped as data.
#
# Precision: matmul operands in fp8e4 with DoubleRow perf mode (2x PE rate;
# contraction pairs packed as [K,2,N] APs), attn-transpose in bf16, softmax
# sum / residual / layernorm in fp32. Attention probabilities are scaled x64
# (to keep them in fp8's normal range) and descaled by 2^-6 at the fused
# residual add.
import sys

for _p in ("/opt/trn_rl_repo", "/root/.axon_site/_ro/trn_rl_repo"):
    if _p not in sys.path:
        sys.path.insert(0, _p)

import numpy as np
import ml_dtypes

import concourse.bass as bass  # noqa: F401  (bass types used via bacc/tile)
import concourse.mybir as mybir
import concourse.tile as tile
from concourse import bacc

BF16 = ml_dtypes.bfloat16
FP8 = ml_dtypes.float8_e4m3
F32 = np.float32

N_CORES = 8
MB = 32
D_MODEL = 1024
D_HALF = 512  # d_content == d_pos
N_HEAD = 8
D_K = 128
DK2 = 64
SCALE = float(D_MODEL) ** 0.5  # 32.0
EPS = 1e-3
P = 128  # partitions
ASCALE = 64.0     # attention-probability scale (fp8 normal range)
DESCALE = 1.0 / ASCALE

DR = mybir.MatmulPerfMode.DoubleRow


def _ceil_to(x, m):
    return (x + m - 1) // m * m


class Plan:
    def __init__(self, lengths):
        lengths = np.asarray(lengths, np.int64)
        assert lengths.shape == (MB,)
        order = np.argsort(-lengths, kind="stable")
        # core c handles sentence ranks {c, 15-c, 16+c, 31-c} (desc length order)
        self.core_sents = [
            [int(order[c]), int(order[15 - c]), int(order[16 + c]), int(order[31 - c])]
            for c in range(N_CORES)
        ]
        self.lengths = lengths
        self.slot_pad = [
            max(int(lengths[self.core_sents[c][j]]) for c in range(N_CORES))
            for j in range(4)
        ]
        self.regions = [_ceil_to(sp, P) for sp in self.slot_pad]
        self.offs = [0]
        for r in self.regions[:-1]:
            self.offs.append(self.offs[-1] + r)
        self.t_pad = sum(self.regions)
        assert self.t_pad % P == 0
        self.nt = self.t_pad // P
        self.glob_off = np.concatenate([[0], np.cumsum(lengths)[:-1]]).astype(np.int64)

    @property
    def key(self):
        return (tuple(self.slot_pad), self.t_pad)


def _copy(nc, eng, out, in_):
    # engine-dispatched copy: DVE has tensor_copy, ACT uses activation(Copy)
    if eng is nc.scalar:
        nc.scalar.copy(out, in_)
    else:
        eng.tensor_copy(out, in_)


def _transpose_qi(nc, at_ps, entry, nk):
    # attn^T @ diag(64*recip): transpose + normalize in one matmul per k-chunk
    attn, diag, lq, qoff = entry
    for ki in range(nk):
        nc.tensor.matmul(
            at_ps[ki][0:P, qoff:qoff + lq],
            attn[0:lq, P * ki:P * ki + P],
            diag[0:lq, 0:lq],
            start=True,
            stop=True,
        )


def _build_program(plan: Plan, loop_n: int = 1):
    """Build and compile the single-core Bass program (same for all cores).

    loop_n > 1 wraps the whole computation in a hardware For-loop (for
    steady-state timing measurements; the body is idempotent)."""
    import contextlib
    T = plan.t_pad
    nc = bacc.Bacc("TRN2", target_bir_lowering=False, debug=False)

    dt = mybir.dt
    # ---- DRAM I/O ----
    # xT packed per-slot: [p, s*(8*gw)] with [p, c, t] = x-dim c*128+p of
    # token t -> one contiguous descriptor per partition per slot
    xT_d = nc.dram_tensor("xT", [P, 8 * T], dt.float8e4, kind="ExternalInput").ap()
    x_d = nc.dram_tensor("x", [T, D_MODEL], dt.float32, kind="ExternalInput").ap()
    # weights packed partition-major on host: one contiguous DMA each
    wq_d = nc.dram_tensor("wq", [P, 4 * 8 * P], dt.float8e4, kind="ExternalInput").ap()
    wk_d = nc.dram_tensor("wk", [P, 4 * 8 * P], dt.float8e4, kind="ExternalInput").ap()
    wv_d = nc.dram_tensor("wv", [P, 8 * D_HALF], dt.float8e4, kind="ExternalInput").ap()
    pw_d = nc.dram_tensor("pw", [P, 8 * D_HALF], dt.float8e4, kind="ExternalInput").ap()
    npad_d = nc.dram_tensor("npad", [P, 4], dt.float32, kind="ExternalInput").ap()
    ident_d = nc.dram_tensor("ident", [P, P], dt.bfloat16, kind="ExternalInput").ap()
    out_d = nc.dram_tensor("out", [T, D_MODEL], dt.float32, kind="ExternalOutput").ap()

    with tile.TileContext(nc) as tc:
        with (
            tc.tile_pool(name="persist", bufs=1) as pp,
            tc.tile_pool(name="weights", bufs=1) as wp,
        ):
            # Per-slot persistent tensors (finer dependency granularity lets
            # attention/proj start as soon as a slot's QKV is done).
            # Q^T/K^T pair-stacked: [p, comp(c/p), pair, region]; partition
            # p<64 holds head 2*pair, p>=64 head 2*pair+1 (comp's 64 dims).
            qt = [pp.tile([P, 2, 4, r], dt.float8e4, name=f"qt{s}", tag=f"qt{s}")
                  for s, r in enumerate(plan.regions)]
            kt = [pp.tile([P, 2, 4, r], dt.float8e4, name=f"kt{s}", tag=f"kt{s}")
                  for s, r in enumerate(plan.regions)]
            # V token-natural, head-major columns: [p, tile, head, {c64|p64}]
            vv = [pp.tile([P, r // P, D_MODEL], dt.float8e4, name=f"vv{s}", tag=f"vv{s}")
                  for s, r in enumerate(plan.regions)]
            o1t = [pp.tile([P, 4, r], dt.float8e4, name=f"o1t{s}", tag=f"o1t{s}")
                   for s, r in enumerate(plan.regions)]
            o2t = [pp.tile([P, 4, r], dt.float8e4, name=f"o2t{s}", tag=f"o2t{s}")
                   for s, r in enumerate(plan.regions)]
            npad_sb = pp.tile([P, 4], dt.float32, tag="npad")
            ident_sb = pp.tile([P, P], dt.bfloat16, tag="ident")

            wq_sb = wp.tile([P, 4, 8, P], dt.float8e4, tag="wq")
            wk_sb = wp.tile([P, 4, 8, P], dt.float8e4, tag="wk")
            wv_sb = wp.tile([P, 8, D_HALF], dt.float8e4, tag="wv")
            pw_sb = wp.tile([P, 2, 4, D_HALF], dt.float8e4, tag="pw")

            _c = getattr(plan, "cfg", {})
            aux_eng = getattr(nc, _c.get("aux_eng", "gpsimd"))
            w_eng = getattr(nc, _c.get("w_eng", "scalar"))
            aux_eng.dma_start(npad_sb[:, :], npad_d[:, :])
            aux_eng.dma_start(ident_sb[:, :], ident_d[:, :])
            # per-pr-chunk weight DMAs on separate queues: the first QK
            # matmuls are gated on a small chunk, not the full weight set
            wq_f = wq_sb.rearrange("p a b c -> p a (b c)")
            wk_f = wk_sb.rearrange("p a b c -> p a (b c)")
            wq_r = wq_d.rearrange("p (a r) -> p a r", a=4)
            wk_r = wk_d.rearrange("p (a r) -> p a r", a=4)
            for pr in range(4):
                w_eng.dma_start(wq_f[:, pr, :], wq_r[:, pr, :])
                w_eng.dma_start(wk_f[:, pr, :], wk_r[:, pr, :])
            aux_eng.dma_start(
                wv_sb.rearrange("p a b -> p (a b)")[:, :], wv_d[:, :])
            aux_eng.dma_start(
                pw_sb.rearrange("p a b c -> p (a b c)")[:, :], pw_d[:, :])

            loop_cm = (tc.For_i(0, loop_n, 1,
                                hint_engines=(mybir.EngineType.PE,
                                              mybir.EngineType.DVE,
                                              mybir.EngineType.Activation,
                                              mybir.EngineType.SP))
                       if loop_n > 1 else contextlib.nullcontext())
            with loop_cm:
                _kernel_body(nc, tc, plan, locals())

    nc.compile()
    return nc


def _kernel_body(nc, tc, plan, env):
    dt = mybir.dt
    qt, kt, vv, o1t, o2t = (env["qt"], env["kt"], env["vv"], env["o1t"],
                            env["o2t"])
    npad_sb, ident_sb = env["npad_sb"], env["ident_sb"]
    wq_sb, wk_sb, wv_sb, pw_sb = (env["wq_sb"], env["wk_sb"], env["wv_sb"],
                                  env["pw_sb"])
    xT_d, x_d, out_d = env["xT_d"], env["x_d"], env["out_d"]
    # dr_*: which matmul groups use fp8 DoubleRow. The board power limiter
    # halves the PE clock under sustained full-DR load, so only the QKV
    # projections (+ the half-array logits, which are power-neutral) and the
    # proj run DoubleRow; attn@V stays plain-fp8. Empirically fastest mix.
    cfg = dict(qk=4, v=3, lg=2, at=1, ot=1, z=6,
               dr_qkv=True, dr_lg=True, dr_av=True, dr_proj=False)
    cfg.update(getattr(plan, "cfg", {}))
    out_eng = getattr(nc, cfg.get("out_eng", "sync"))
    aux_eng = getattr(nc, cfg.get("aux_eng", "gpsimd"))
    ms_gp = cfg.get("ms_gp", True)
    ln_gp = cfg.get("ln_gp", True)

    # zero the attention-output staging (pad-query columns are never
    # written; keep them finite for the projection matmuls)
    ms_eng = nc.gpsimd if ms_gp else nc.vector
    for s in range(4):
        L, r = plan.slot_pad[s], plan.regions[s]
        if L < r:
            ms_eng.memset(o1t[s][:, :, L:r], 0.0)
            ms_eng.memset(o2t[s][:, :, L:r], 0.0)

    # ============ Phase 1: QKV projection emitters ============
    # Only slot 0 runs as a dense up-front phase; slots 1-3's QKV matmuls
    # are interleaved as PE filler into the previous slot's attention so
    # the PE never idles long enough for the HAM to re-throttle its clock.
    dr_qkv = cfg.get("dr_qkv", True)

    def _xt_load(xtp, s):
        gw = plan.regions[s]
        g0 = plan.offs[s]
        xt_sb = xtp.tile([P, 8, gw], dt.float8e4, name="xt", tag="xt")
        xt_f2 = xt_sb.rearrange("p a b -> p (a b)")
        # split across several DMA queues (1-1.5KB/partition descriptors)
        for c in range(4):
            nc.sync.dma_start(
                xt_f2[:, 2 * c * gw:2 * (c + 1) * gw],
                xT_d[:, 8 * g0 + 2 * c * gw:8 * g0 + 2 * (c + 1) * gw])
        return xt_sb

    def _qkv_emitters(pool, s, xt_sb, pr_major=False):
        """Closures, each emitting one PSUM acc group (matmuls + copy)."""
        gw = plan.regions[s]
        ems = []

        def _qk_group(half, pr, qk):
            w_sb, dst = ((wq_sb, qt), (wk_sb, kt))[qk]
            acc = pool.tile([P, 512], dt.float32, name="qkacc", tag="lg")
            if dr_qkv:
                for jj in range(2):
                    j0 = half * 4 + 2 * jj
                    nc.tensor.matmul(
                        acc[:, 0:gw],
                        w_sb[:, pr, j0:j0 + 2, :],
                        xt_sb[:, j0:j0 + 2, 0:gw],
                        start=(jj == 0),
                        stop=(jj == 1),
                        perf_mode=DR,
                    )
            else:
                for jj in range(4):
                    j = half * 4 + jj
                    nc.tensor.matmul(
                        acc[:, 0:gw],
                        w_sb[:, pr, j, :],
                        xt_sb[:, j, 0:gw],
                        start=(jj == 0),
                        stop=(jj == 3),
                    )
            eng = nc.vector if (pr + qk + half) % 2 else nc.scalar
            _copy(nc, eng, dst[s][:, half, pr, 0:gw], acc[:, 0:gw])

        vv_w = vv[s].rearrange("p t (h b d) -> p t h b d", h=N_HEAD, b=2)

        def _v_group(tt, half):
            tl = tt * P
            vacc = pool.tile([P, 512], dt.float32, name="vacc", tag="lg")
            vacc_r = vacc.rearrange("p (h d) -> p h d", h=N_HEAD)
            if dr_qkv:
                for jj in range(2):
                    j0 = half * 4 + 2 * jj
                    nc.tensor.matmul(
                        vacc[:, :],
                        xt_sb[:, j0:j0 + 2, tl:tl + P],
                        wv_sb[:, j0:j0 + 2, :],
                        start=(jj == 0),
                        stop=(jj == 1),
                        perf_mode=DR,
                    )
            else:
                for jj in range(4):
                    j = half * 4 + jj
                    nc.tensor.matmul(
                        vacc[:, :],
                        xt_sb[:, j, tl:tl + P],
                        wv_sb[:, j, :],
                        start=(jj == 0),
                        stop=(jj == 3),
                    )
            nc.scalar.copy(vv_w[:, tt, :, half, :], vacc_r[:, :, :])

        if pr_major:
            # up-front slot: pr-major order so head 0's q/k (pr=0, both
            # halves) complete first and attention starts earliest
            for pr in range(4):
                for half in range(2):
                    for qk in range(2):
                        ems.append(lambda h=half, p=pr, q=qk: _qk_group(h, p, q))
            for half in range(2):
                for tt in range(gw // P):
                    ems.append(lambda t=tt, h=half: _v_group(t, h))
        else:
            # half-0 groups first: PE can start while xt half-1 is landing
            for half in range(2):
                for pr in range(4):
                    for qk in range(2):
                        ems.append(lambda h=half, p=pr, q=qk: _qk_group(h, p, q))
                for tt in range(gw // P):
                    ems.append(lambda t=tt, h=half: _v_group(t, h))
        return ems

    # dense up-front QKV for slot 0 in its own (wider) PSUM scope
    with (
        tc.tile_pool(name="xt0_pool", bufs=1) as xtp0,
        tc.tile_pool(name="p1_ps", bufs=7, space="PSUM") as p1ps,
    ):
        xt0 = _xt_load(xtp0, 0)
        for em in _qkv_emitters(p1ps, 0, xt0, pr_major=True):
            em()

    # ======== Phase 2+3: attention + proj/LN + QKV filler ========
    # PSUM: lg/filler(2) + atp0..3(4) + ozp(2, attn-out + proj acc) = 8
    x_dma = nc.sync
    out_dma = out_eng
    with (
        tc.tile_pool(name="xt_pool", bufs=2) as xtp,
        tc.tile_pool(name="lg_ps", bufs=2, space="PSUM") as lgps,
        tc.tile_pool(name="at_ps", bufs=1, space="PSUM") as atps,
        tc.tile_pool(name="ozp_ps", bufs=2, space="PSUM") as ozps,
        tc.tile_pool(name="attn_sb", bufs=6) as asb,
        tc.tile_pool(name="small_sb", bufs=8) as ssb,
        tc.tile_pool(name="z_sb", bufs=3) as zsb,
        tc.tile_pool(name="x_sb", bufs=6) as xsb,
        tc.tile_pool(name="ln_sb", bufs=4) as lsb,
    ):
        def _slot_dims(s):
            gw = plan.regions[s]
            L = plan.slot_pad[s]
            nk = gw // P
            return L, gw, nk

        p3_state = {}  # slot -> (z tiles, mv4 tile)

        def _emit_p3a(s, tt):
            # proj + residual + one-pass LN stats for one token tile
            g0 = plan.offs[s]
            lt = tt * P
            t0 = g0 + lt
            if tt == 0:
                ntt0 = plan.regions[s] // P
                xfs = []
                for t2 in range(ntt0):
                    xf = xsb.tile([P, D_MODEL], dt.float32, name="xf",
                                  tag="xf", bufs=6)
                    x_dma.dma_start(xf[:, :], x_d[g0 + t2 * P:g0 + (t2 + 1) * P, :])
                    xfs.append(xf)
                p3_state[s] = ([None] * 4,
                               lsb.tile([P, 4, 2], dt.float32, name="mv4", tag="mv4",
                                        bufs=2),
                               xfs)
            zs, mv4, xfs = p3_state[s]
            zh = []
            for i, osrc in enumerate((o1t[s], o2t[s])):
                zp = ozps.tile([P, 512], dt.float32, name="zp", tag="ozp")
                if cfg.get("dr_proj", True):
                    for kp in range(2):
                        nc.tensor.matmul(
                            zp[:, :],
                            osrc[:, 2 * kp:2 * kp + 2, lt:lt + P],
                            pw_sb[:, i, 2 * kp:2 * kp + 2, :],
                            start=(kp == 0),
                            stop=(kp == 1),
                            perf_mode=DR,
                        )
                else:
                    for k in range(4):
                        nc.tensor.matmul(
                            zp[:, :],
                            osrc[:, k, lt:lt + P],
                            pw_sb[:, i, k, :],
                            start=(k == 0),
                            stop=(k == 3),
                        )
                zh.append(zp)
            xt_f = xfs[tt]
            z = zsb.tile([P, D_MODEL], dt.float32, tag="z", bufs=5)
            zs[tt] = z
            # z = zp + 64*x (whole chain is x64; exactly undone via the
            # x64-scaled EPS in the LN)
            for i in range(2):
                nc.vector.tensor_tensor(
                    z[:, i * D_HALF:(i + 1) * D_HALF],
                    zh[i][:, :],
                    xt_f[:, i * D_HALF:(i + 1) * D_HALF],
                    mybir.AluOpType.add,
                )
            # one-pass LN statistics on DVE (BN hardware): 2 subgroups of 512
            stats = lsb.tile([P, 2, 6], dt.float32, name="bnst", tag="bnst")
            for i in range(2):
                nc.vector.bn_stats(
                    out=stats[:, i, :],
                    in_=z[:, i * D_HALF:(i + 1) * D_HALF],
                )
            nc.vector.bn_aggr(out=mv4[:, tt, :], in_=stats[:, :, :])

        def _emit_p3b(s):
            # batched LN tail for the whole slot: one sqrt (one ACT table
            # swap pair per slot instead of per tile)
            gw = plan.regions[s]
            g0 = plan.offs[s]
            ntt = gw // P
            zs, mv4, _ = p3_state.pop(s)
            sig4 = lsb.tile([P, 4], dt.float32, name="sig4", tag="sig4", bufs=2)
            # unbiased sigma from biased bn variance: sqrt(var * N/(N-1))
            nc.scalar.activation(
                sig4[:, 0:ntt], mv4[:, 0:ntt, 1],
                mybir.ActivationFunctionType.Sqrt,
                scale=float(D_MODEL) / (D_MODEL - 1),
            )
            # the whole z chain is x64 (ident64 attention scale + x64
            # residual); scaling EPS by 64 keeps the LN output exact
            nc.gpsimd.tensor_scalar(
                sig4[:, 0:ntt], sig4[:, 0:ntt], ASCALE * EPS, None,
                mybir.AluOpType.add,
            )
            for tt in range(ntt):
                # scalars must be contiguous [P, 1] tiles: strided scalar
                # APs drop GpSimd's tensor_scalar into a ~15us slow path
                negmu = lsb.tile([P, 1], dt.float32, name="negmu",
                                 tag="negmu", bufs=2)
                nc.gpsimd.tensor_scalar(
                    negmu[:, :], mv4[:, tt, 0:1], -1.0, None,
                    mybir.AluOpType.mult,
                )
                rstd = lsb.tile([P, 1], dt.float32, name="rstd",
                                tag="rstd", bufs=2)
                nc.vector.reciprocal(rstd[:, :], sig4[:, tt:tt + 1])
                o = zsb.tile([P, D_MODEL], dt.float32, tag="o")
                eng_o = nc.gpsimd if ln_gp else nc.vector
                eng_o.tensor_scalar(
                    o[:, :], zs[tt][:, :], negmu[:, :], rstd[:, :],
                    mybir.AluOpType.add, mybir.AluOpType.mult,
                )
                out_dma.dma_start(out_d[g0 + tt * P:g0 + (tt + 1) * P, :],
                                  o[:, :])

        def _stage_prev(prev):
            # stage prev head's normalized (x64) attn^T slabs PSUM->SBUF as
            # fp8 DoubleRow chunk-pairs; odd tail chunk staged single
            ps, pot, pat, ph = prev
            L, gw, nk = _slot_dims(ps)
            # one staging copy for the whole head: [P, nk, L]
            ab = asb.tile([P, 4, 512], dt.float8e4, name="at_sb4",
                          tag="at_sb4", bufs=4)
            eng = nc.vector if ph % 2 else nc.scalar
            _copy(nc, eng, ab[:, 0:nk, 0:L], pat[:, 0:nk, 0:L])
            sbs = []
            if cfg.get("dr_av", True):
                for kp in range(nk // 2):
                    sbs.append((ab, True, 2 * kp))
                if nk % 2:
                    sbs.append((ab, False, nk - 1))
            else:
                for ki in range(nk):
                    sbs.append((ab, False, ki))
            return sbs

        def _prev_ot(prev, sbs, idx):
            # one attn@V chunk(-pair) of the prev head, streamed under the
            # current head's logits
            ps, pot, pat, ph = prev
            L, gw, nk = _slot_dims(ps)
            pvv = vv[ps].rearrange("p t (h d) -> p t h d", h=N_HEAD)
            ab, is_pair, k0 = sbs[idx]
            last = idx == len(sbs) - 1
            if is_pair:
                nc.tensor.matmul(
                    pot[:, 0:L],
                    pvv[0:P, k0:k0 + 2, ph, :],
                    ab[:, k0:k0 + 2, 0:L],
                    start=(idx == 0),
                    stop=last,
                    perf_mode=DR,
                )
            else:
                nc.tensor.matmul(
                    pot[:, 0:L],
                    pvv[0:P, k0, ph, :],
                    ab[:, k0, 0:L],
                    start=(idx == 0),
                    stop=last,
                )

        def _prev_out(prev):
            ps, pot, pat, ph = prev
            L, gw, nk = _slot_dims(ps)
            php, ppr = 64 * (ph % 2), ph // 2
            nc.vector.tensor_copy(
                o1t[ps][php:php + 64, ppr, 0:L], pot[0:64, 0:L])
            nc.vector.tensor_copy(
                o2t[ps][php:php + 64, ppr, 0:L], pot[64:128, 0:L])

        prev = None       # (slot, ot_psum, at_ps, head) not yet V-multiplied
        pend_p3 = None    # slot whose proj/LN is not yet emitted
        fill_q = []       # pending QKV emitters for the next slot
        for s in range(4):
            L, gw, nk = _slot_dims(s)
            nq = nk
            if s + 1 < 4:
                xt_next = _xt_load(xtp, s + 1)
                fill_q = _qkv_emitters(lgps, s + 1, xt_next)
            for h in range(N_HEAD):
                half, pr = h % 2, h // 2
                hp = 64 * half
                # attn^T slabs per key-chunk: [kc, all slot queries] so the
                # attn@V contraction runs one N=L matmul per chunk
                at_all = atps.tile([P, 4, 512], dt.float32, name="atp",
                                   tag="atp")
                at_ps = [at_all[:, ki, :] for ki in range(nk)]
                ot = ozps.tile([P, 512], dt.float32, name="ot", tag="ozp")
                sbs = _stage_prev(prev) if prev is not None else None
                nprev = len(sbs) if prev is not None else 0
                pend = {}
                for qi in range(nq):
                    qoff = P * qi
                    lq = min(P, L - P * qi)
                    lg = lgps.tile([P, 512], dt.float32, tag="lg")
                    # logits over the full padded region: pad keys are exact
                    # zeros -> exp contributes 1, corrected via npad
                    if cfg.get("dr_lg", True):
                        nc.tensor.matmul(
                            lg[0:lq, 0:gw],
                            qt[s][hp:hp + 64, :, pr, qoff:qoff + lq],
                            kt[s][hp:hp + 64, :, pr, 0:gw],
                            start=True,
                            stop=True,
                            perf_mode=DR,
                        )
                    else:
                        for comp in range(2):
                            nc.tensor.matmul(
                                lg[0:lq, 0:gw],
                                qt[s][hp:hp + 64, comp, pr, qoff:qoff + lq],
                                kt[s][hp:hp + 64, comp, pr, 0:gw],
                                start=(comp == 0),
                                stop=(comp == 1),
                            )
                    if prev is not None and qi < nprev:
                        _prev_ot(prev, sbs, qi)
                    # one next-slot QKV group per section: keeps the PE
                    # dense so the HAM clock never drops to 4/8
                    if fill_q:
                        fill_q.pop(0)()
                    attn = asb.tile([P, 512], dt.bfloat16, tag="attn")
                    se = ssb.tile([P, 1], dt.float32, tag="se")
                    # exp only over [0:L]; region-pad columns memset to zero
                    # so the transposes read exact zeros there
                    if L < gw:
                        nc.gpsimd.memset(attn[0:P, L:gw], 0.0)
                    nc.scalar.activation(
                        attn[0:lq, 0:L],
                        lg[0:lq, 0:L],
                        mybir.ActivationFunctionType.Exp,
                        scale=1.0 / SCALE,
                        accum_out=se[0:lq, :],
                    )
                    rc = ssb.tile([P, 1], dt.float32, tag="rc")
                    nc.gpsimd.tensor_tensor(
                        rc[0:lq, :], se[0:lq, :], npad_sb[0:lq, s:s + 1],
                        mybir.AluOpType.subtract,
                    )
                    nc.vector.reciprocal(rc[0:lq, :], rc[0:lq, :])
                    diag = ssb.tile([P, P], dt.bfloat16, tag="diag")
                    # ident is 64*I: diag = 64/denominator
                    nc.vector.tensor_scalar(
                        diag[0:lq, 0:lq], ident_sb[0:lq, 0:lq],
                        rc[0:lq, :], None, mybir.AluOpType.mult,
                    )
                    # transposes deferred two logits back: the
                    # exp->rc->diag chain gets ~2 sections of slack
                    if qi >= 2:
                        _transpose_qi(nc, at_ps, pend[qi - 2], nk)
                    pend[qi] = (attn, diag, lq, qoff)
                # flush: prev head's remaining ot chunks, last transposes
                if prev is not None:
                    for ki in range(nq, nprev):
                        _prev_ot(prev, sbs, ki)
                for qf in range(max(0, nq - 2), nq):
                    _transpose_qi(nc, at_ps, pend[qf], nk)
                if prev is not None:
                    _prev_out(prev)
                prev = (s, ot, at_all, h)
                if pend_p3 is not None and 3 <= h <= 6:
                    if h - 3 < plan.regions[pend_p3] // P:
                        _emit_p3a(pend_p3, h - 3)
                    if h == 6:
                        _emit_p3b(pend_p3)
                        pend_p3 = None
            # next slot's QKV must be complete before its attention starts
            while fill_q:
                fill_q.pop(0)()
            pend_p3 = s
        # drain the final head and the last slot's proj/LN
        sbs = _stage_prev(prev)
        for ki in range(len(sbs)):
            _prev_ot(prev, sbs, ki)
        _prev_out(prev)
        for tt in range(plan.regions[3] // P):
            _emit_p3a(3, tt)
        _emit_p3b(3)


_PROGRAMS = {}   # plan.key -> (nc, plan)
_RUNNERS = {}    # plan.key -> callable(in_maps) -> list[dict]


def _get_program(plan: Plan):
    if plan.key not in _PROGRAMS:
        _PROGRAMS[plan.key] = _build_program(plan)
    return _PROGRAMS[plan.key]


def _make_runner(nc, donate=True):
    """Cached PJRT runner (mirrors bass_utils.run_bass_kernel_spmd's axon
    path via bass2jax, but reuses the jitted executable across calls)."""
    import jax
    from jax.sharding import Mesh, PartitionSpec
    from jax.experimental.shard_map import shard_map
    from concourse import bass2jax

    bass2jax.install_neuronx_cc_hook()

    partition_name = (nc.partition_id_tensor.name
                      if nc.partition_id_tensor else None)
    in_names, out_names, out_avals, zero_shapes = [], [], [], []
    for alloc in nc.m.functions[0].allocations:
        if not isinstance(alloc, mybir.MemoryLocationSet):
            continue
        name = alloc.memorylocations[0].name
        if alloc.kind == "ExternalInput":
            if name == partition_name:
                continue
            in_names.append(name)
        elif alloc.kind == "ExternalOutput":
            out_names.append(name)
            shape = tuple(alloc.tensor_shape)
            dtype = mybir.dt.np(alloc.dtype)
            out_avals.append(jax.core.ShapedArray(shape, dtype))
            zero_shapes.append((shape, dtype))
    n_params = len(in_names)
    all_names = in_names + out_names
    if partition_name is not None:
        all_names = all_names + [partition_name]

    def _body(*args):
        operands = list(args)
        if partition_name is not None:
            operands.append(bass2jax.partition_id_tensor())
        outs = bass2jax._bass_exec_p.bind(
            *operands,
            out_avals=tuple(out_avals),
            in_names=tuple(all_names),
            out_names=tuple(out_names),
            lowering_input_output_aliases=(),
            sim_require_finite=True,
            sim_require_nnan=True,
            nc=nc,
        )
        return tuple(outs)

    devices = jax.devices()[:N_CORES]
    mesh = Mesh(np.asarray(devices), ("core",))
    in_specs = (PartitionSpec("core"),) * (n_params + len(out_names))
    out_specs = (PartitionSpec("core"),) * len(out_names)
    sharded = jax.jit(
        shard_map(_body, mesh=mesh, in_specs=in_specs, out_specs=out_specs,
                  check_rep=False),
        donate_argnums=tuple(range(n_params, n_params + len(out_names)))
        if donate else (),
        keep_unused=True,
    )

    def run(in_maps):
        concat_in = [
            np.concatenate([np.asarray(m[name]) for m in in_maps], axis=0)
            for name in in_names
        ]
        concat_zeros = [
            np.zeros((N_CORES * s[0], *s[1:]), d) for (s, d) in zero_shapes
        ]
        out_arrs = sharded(*concat_in, *concat_zeros)
        return [
            {
                name: np.asarray(out_arrs[i]).reshape(
                    N_CORES, *out_avals[i].shape)[c]
                for i, name in enumerate(out_names)
            }
            for c in range(N_CORES)
        ]

    run.sharded = sharded
    run.in_names = in_names
    run.out_names = out_names
    run.out_avals = out_avals
    run.zero_shapes = zero_shapes
    run.n_params = n_params
    return run


def _prep_weights(w_qs1, w_ks1, w_vs1, w_qs2, w_ks2, w_vs2, proj1_w, proj2_w):
    wq, wk, wv, pw = _prep_weights_4d(w_qs1, w_ks1, w_vs1, w_qs2, w_ks2,
                                      w_vs2, proj1_w, proj2_w)
    # partition-major packing: one contiguous DMA per weight tensor on device
    wq = np.ascontiguousarray(wq.transpose(2, 0, 1, 3).reshape(P, -1))
    wk = np.ascontiguousarray(wk.transpose(2, 0, 1, 3).reshape(P, -1))
    wv = np.ascontiguousarray(wv.transpose(1, 0, 2).reshape(P, -1))
    pw = np.ascontiguousarray(pw.transpose(2, 0, 1, 3).reshape(P, -1))
    return wq, wk, wv, pw


def _prep_weights_4d(w_qs1, w_ks1, w_vs1, w_qs2, w_ks2, w_vs2, proj1_w, proj2_w):
    wq = np.zeros((4, 8, P, P), FP8)
    wk = np.zeros((4, 8, P, P), FP8)
    for pr in range(4):
        h0, h1 = 2 * pr, 2 * pr + 1
        for j in range(8):
            if j < 4:
                rows = slice(j * P, (j + 1) * P)
                wq[pr, j] = np.concatenate(
                    [w_qs1[h0, rows, :], w_qs1[h1, rows, :]], axis=1).astype(FP8)
                wk[pr, j] = np.concatenate(
                    [w_ks1[h0, rows, :], w_ks1[h1, rows, :]], axis=1).astype(FP8)
            else:
                rows = slice((j - 4) * P, (j - 3) * P)
                wq[pr, j] = np.concatenate(
                    [w_qs2[h0, rows, :], w_qs2[h1, rows, :]], axis=1).astype(FP8)
                wk[pr, j] = np.concatenate(
                    [w_ks2[h0, rows, :], w_ks2[h1, rows, :]], axis=1).astype(FP8)
    wv = np.zeros((8, P, D_HALF), FP8)
    for j in range(8):
        src = w_vs1 if j < 4 else w_vs2
        rows = slice((j % 4) * P, (j % 4 + 1) * P)
        wv[j] = np.concatenate([src[h, rows, :] for h in range(8)], axis=1
                               ).astype(FP8)
    pw = np.zeros((2, 4, P, D_HALF), FP8)
    p1T = np.ascontiguousarray(proj1_w.T)  # [in, out]
    p2T = np.ascontiguousarray(proj2_w.T)
    for k in range(4):
        pw[0, k] = p1T[k * P:(k + 1) * P, :].astype(FP8)
        pw[1, k] = p2T[k * P:(k + 1) * P, :].astype(FP8)
    return wq, wk, wv, pw


def _prep_core_inputs(plan: Plan, inp, c):
    T = plan.t_pad
    x = np.zeros((T, D_MODEL), F32)
    npad = np.zeros((4,), F32)
    for j in range(4):
        s = plan.core_sents[c][j]
        L = int(plan.lengths[s])
        g0 = int(plan.glob_off[s])
        x[plan.offs[j]:plan.offs[j] + L] = inp[g0:g0 + L]
        # exp runs over [0, slot_pad); every pad key contributes
        # exp(0)=1 to the softmax denominator
        npad[j] = plan.slot_pad[j] - L
    # per-slot packed transpose: [p, c, t] = x[t, c*128+p], slots contiguous
    xT = np.zeros((P, 8 * T), FP8)
    for j in range(4):
        gw, g0 = plan.regions[j], plan.offs[j]
        blk = x[g0:g0 + gw].T.reshape(8, P, gw).transpose(1, 0, 2)
        xT[:, 8 * g0:8 * (g0 + gw)] = blk.reshape(P, 8 * gw).astype(FP8)
    npad_rep = np.tile(npad[None, :], (P, 1)).astype(F32)
    # residual ships pre-scaled x64 to match the x64 attention chain
    # (exact: power-of-two scale, undone via the x64-scaled LN epsilon)
    return x * ASCALE, xT, npad_rep


def make_in_maps(plan: Plan, inp, weights):
    wq, wk, wv, pw = weights
    ident = (np.eye(P) * ASCALE).astype(BF16)
    in_maps = []
    for c in range(N_CORES):
        x, xT, npad_rep = _prep_core_inputs(plan, inp, c)
        in_maps.append({
            "xT": xT, "x": x, "wq": wq, "wk": wk, "wv": wv, "pw": pw,
            "npad": npad_rep, "ident": ident,
        })
    return in_maps


def gather_output(plan: Plan, results, a_2=None, b_2=None):
    T_tot = int(plan.lengths.sum())
    out = np.empty((T_tot, D_MODEL), F32)
    for c in range(N_CORES):
        oc = results[c]["out"]
        for j in range(4):
            s = plan.core_sents[c][j]
            L = int(plan.lengths[s])
            g0 = int(plan.glob_off[s])
            out[g0:g0 + L] = oc[plan.offs[j]:plan.offs[j] + L]
    if a_2 is not None and (np.any(a_2 != 1.0) or np.any(b_2 != 0.0)):
        out = out * np.asarray(a_2, F32) + np.asarray(b_2, F32)
    return out


def kernel(inp, w_qs1, w_ks1, w_vs1, w_qs2, w_ks2, w_vs2,
           proj1_w, proj2_w, a_2, b_2, token_batch, token_pos, valid_mask):
    inp = np.asarray(inp, F32)
    token_batch = np.asarray(token_batch)
    lengths = np.bincount(token_batch, minlength=MB).astype(np.int64)
    # tokens of each sentence must be contiguous and in order
    plan = Plan(lengths)

    nc = _get_program(plan)
    if plan.key not in _RUNNERS:
        _RUNNERS[plan.key] = _make_runner(nc)
    runner = _RUNNERS[plan.key]

    weights = _prep_weights(np.asarray(w_qs1), np.asarray(w_ks1),
                            np.asarray(w_vs1), np.asarray(w_qs2),
                            np.asarray(w_ks2), np.asarray(w_vs2),
                            np.asarray(proj1_w), np.asarray(proj2_w))
    in_maps = make_in_maps(plan, inp, weights)
    results = runner(in_maps)
    return gather_output(plan, results, np.asarray(a_2), np.asarray(b_2))
